# revision 49
# baseline (speedup 1.0000x reference)
"""Multi-head attention Trainium2 kernel (B=8,S=1024,D=1024,H=16,DK=64).

Data-parallel over batch: one batch element per NeuronCore (8 cores).

Modes:
  f32    - exact baseline (DRAM-scratch structure, fp32 matmuls)
  f32r   - same structure, float32r matmuls
  bf16   - same structure, bf16 matmuls + bf16 scratch
  bf16v2 - restructured: projections emit transposed per-head layouts
           directly into SBUF (no q/k/o DRAM round trips), exp packed
           into 9x[128,512] chunks/head, one-head software pipeline.
  bf16v3 - default. v2 plus: every load is a gpsimd cast-DMA (f32 DRAM
           -> bf16 SBUF, no staging copies, half the modeled DMA time);
           bf16 input transposes; the head loop starts right after the
           Q/K ch0 projections with the remaining projection work
           (Q/K ch1, V m>=2, wo load) paced into the head loop via a
           deferred-work queue tuned so the PE stays fed through the
           ACT(exp)-bound late heads; causal masks on DVE; PV psum
           double-buffered; merged single-psum otrans; finals split
           into half-m chunks spread one-per-head; split evac + dual
           store queues on the last final to shorten the drain.
"""

import numpy as np

import concourse.bass as bass
import concourse.mybir as mybir
import concourse.tile as tile
from concourse import bacc
from concourse.bass_utils import run_bass_kernel_spmd
from concourse.masks import make_identity

B, S, D, H, DK = 8, 1024, 1024, 16, 64
P = 128
F32 = mybir.dt.float32
F32R = mybir.dt.float32r
BF16 = mybir.dt.bfloat16


HP_BUFS = 3


def _build_nc(mm_mode: str = "f32"):
    """Build the Bass program. mm_mode: 'f32' (exact), 'f32r' (fast fp32),
    or 'bf16' (all matmuls + DRAM scratch in bfloat16)."""

    if mm_mode == "bf16":
        MDT = BF16
        SDT = BF16  # DRAM scratch + per-head staging dtype
    else:
        MDT = F32R if mm_mode == "f32r" else F32
        SDT = F32

    def mmc(ap):
        return ap

    nc = bacc.Bacc(
        "TRN2",
        target_bir_lowering=False,
        debug=False,
        enable_asserts=False,
        num_devices=B,
    )

    q_d = nc.dram_tensor("q", [S, D], F32, kind="ExternalInput")
    k_d = nc.dram_tensor("k", [S, D], F32, kind="ExternalInput")
    v_d = nc.dram_tensor("v", [S, D], F32, kind="ExternalInput")
    wq_d = nc.dram_tensor("w_q", [D, D], F32, kind="ExternalInput")
    wk_d = nc.dram_tensor("w_k", [D, D], F32, kind="ExternalInput")
    wv_d = nc.dram_tensor("w_v", [D, D], F32, kind="ExternalInput")
    wo_d = nc.dram_tensor("w_o", [D, D], F32, kind="ExternalInput")
    out_d = nc.dram_tensor("out", [S, D], F32, kind="ExternalOutput")

    qp_d = nc.dram_tensor("qp_scratch", [S, D], SDT, kind="Internal")
    kp_d = nc.dram_tensor("kp_scratch", [S, D], SDT, kind="Internal")
    vp_d = nc.dram_tensor("vp_scratch", [S, D], SDT, kind="Internal")
    op_d = nc.dram_tensor("op_scratch", [S, D], SDT, kind="Internal")

    with tile.TileContext(nc) as tc:
        with (
            tc.tile_pool(name="consts", bufs=1) as consts,
            tc.tile_pool(name="wpool", bufs=2) as wpool,
            tc.tile_pool(name="xtp", bufs=1) as xtp,
            tc.tile_pool(name="iop", bufs=3) as iop,
            tc.tile_pool(name="shp", bufs=3) as shp,
            tc.tile_pool(name="hp", bufs=HP_BUFS) as hp,
            tc.tile_pool(name="ptp", bufs=3) as ptp,
            tc.tile_pool(name="sp", bufs=4) as sp,
            tc.tile_pool(name="psA", bufs=3, space="PSUM") as psA,
            tc.tile_pool(name="psB", bufs=2, space="PSUM") as psB,
            tc.tile_pool(name="psT", bufs=2, space="PSUM") as psT,
            tc.tile_pool(name="psC", bufs=2, space="PSUM") as psC,
        ):
            ident = consts.tile([P, P], F32, tag="ident")
            make_identity(nc, ident[:])
            if SDT == BF16:
                identb = consts.tile([P, P], BF16, tag="identb")
                make_identity(nc, identb[:])
            else:
                identb = ident
            # tri[k, q] = 1.0 if q >= k else 0.0  (keep causal-valid entries)
            tri = consts.tile([P, P], SDT, tag="tri")
            nc.gpsimd.memset(tri[:], 1.0)
            nc.gpsimd.affine_select(
                out=tri[:],
                in_=tri[:],
                compare_op=mybir.AluOpType.is_ge,
                fill=0.0,
                base=0,
                pattern=[[1, P]],
                channel_multiplier=-1,
            )

            # ---------------- Phase 1: projections -> DRAM scratch ----------
            for x_d, w_d, xp_d in (
                (q_d, wq_d, qp_d),
                (k_d, wk_d, kp_d),
                (v_d, wv_d, vp_d),
            ):
                xt_sb = xtp.tile([P, 8, 1024], MDT, tag="xt")
                for st in range(8):
                    nat = iop.tile([P, 1024], F32, tag="nat")
                    nc.sync.dma_start(nat[:], x_d.ap()[st * P : (st + 1) * P, :])
                    for kd in range(8):
                        tp = psB.tile([P, P], F32, tag="tp")
                        nc.tensor.transpose(
                            tp[:], nat[:, kd * P : (kd + 1) * P], ident[:]
                        )
                        nc.vector.tensor_copy(
                            out=xt_sb[:, kd, st * P : (st + 1) * P], in_=tp[:]
                        )
                for ch in range(2):
                    w_sb = wpool.tile([P, 8, 512], MDT, tag="w")
                    wsrc = w_d.ap()[:, ch * 512 : (ch + 1) * 512]
                    if MDT == F32:
                        nc.sync.dma_start(
                            w_sb[:], wsrc.rearrange("(kd p) c -> p kd c", p=P)
                        )
                    else:
                        for kd in range(8):
                            wstg = iop.tile([P, 512], F32, tag="wstg")
                            nc.sync.dma_start(
                                wstg[:], wsrc[kd * P : (kd + 1) * P, :]
                            )
                            nc.scalar.copy(out=w_sb[:, kd, :], in_=wstg[:])
                    for st in range(8):
                        ps = psA.tile([P, 512], F32, tag="mm")
                        for kd in range(8):
                            nc.tensor.matmul(
                                ps[:],
                                mmc(xt_sb[:, kd, st * P : (st + 1) * P]),
                                mmc(w_sb[:, kd, :]),
                                start=(kd == 0),
                                stop=(kd == 7),
                            )
                        stg = iop.tile([P, 512], SDT, tag="stg")
                        nc.vector.tensor_copy(out=stg[:], in_=ps[:])
                        nc.scalar.dma_start(
                            xp_d.ap()[
                                st * P : (st + 1) * P, ch * 512 : (ch + 1) * 512
                            ],
                            stg[:],
                        )

            # ------------- Phase 2: attention, one head at a time -----------
            qp_r = qp_d.ap().rearrange("(h a) (b u) -> h (a b) u", h=H, b=16)
            kp_r = kp_d.ap().rearrange("(h a) (b u) -> h (a b) u", h=H, b=16)
            vp_r = vp_d.ap().rearrange("(h a) (b u) -> h (a b) u", h=H, b=16)
            op_w = op_d.ap().rearrange(
                "(hh i pa) (pb u) -> hh pa pb i u", i=8, pa=8, pb=16
            )

            for hp2 in range(H // 2):
                h0 = 2 * hp2
                qT2 = hp.tile([P, 1024], MDT, tag="qhT")
                kT2 = hp.tile([P, 1024], MDT, tag="khT")
                qh2 = shp.tile([P, 8, P], SDT, tag="qh")
                kh2 = shp.tile([P, 8, P], SDT, tag="kh")
                for hh in range(2):
                    nc.sync.dma_start(
                        qh2[:, :, hh * DK : (hh + 1) * DK],
                        qp_r[h0 + hh].rearrange("(t p) u -> p t u", p=P),
                    )
                    nc.scalar.dma_start(
                        kh2[:, :, hh * DK : (hh + 1) * DK],
                        kp_r[h0 + hh].rearrange("(t p) u -> p t u", p=P),
                    )
                for t in range(8):
                    tpq = psT.tile([P, P], SDT, tag="tph")
                    nc.tensor.transpose(tpq[:], qh2[:, t, :], identb[:])
                    nc.vector.tensor_copy(
                        out=qT2[:, t * P : (t + 1) * P], in_=tpq[:]
                    )
                    tpk = psT.tile([P, P], SDT, tag="tph")
                    nc.tensor.transpose(tpk[:], kh2[:, t, :], identb[:])
                    nc.vector.tensor_copy(
                        out=kT2[:, t * P : (t + 1) * P], in_=tpk[:]
                    )

                for hh in range(2):
                    h = h0 + hh
                    r0, r1 = hh * DK, (hh + 1) * DK
                    vo = hp.tile([P, 8, DK + 1], SDT, tag="vo")
                    if h < HP_BUFS:
                        nc.vector.memset(vo[:, :, DK : DK + 1], 1.0)
                    nc.gpsimd.dma_start(
                        vo[:, :, :DK], vp_r[h].rearrange("(t p) u -> p t u", p=P)
                    )

                    pt = ptp.tile([P, 4608], SDT, tag="pt")
                    ptoff = [j * 1024 - 64 * j * (j - 1) for j in range(9)]
                    for j in range(8):
                        q0 = j * P
                        off = q0
                        while off < 1024:
                            n = min(512, 1024 - off)
                            ps = psA.tile([P, 512], F32, tag="mm")
                            nc.tensor.matmul(
                                ps[:, :n],
                                mmc(kT2[r0:r1, q0 : q0 + P]),
                                mmc(qT2[r0:r1, off : off + n]),
                                start=True,
                                stop=True,
                            )
                            nc.scalar.activation(
                                out=pt[:, ptoff[j] + off - q0 : ptoff[j] + off - q0 + n],
                                in_=ps[:, :n],
                                func=mybir.ActivationFunctionType.Exp,
                                scale=0.125,
                            )
                            off += n
                        nc.vector.tensor_tensor(
                            pt[:, ptoff[j] : ptoff[j] + P],
                            pt[:, ptoff[j] : ptoff[j] + P],
                            tri[:],
                            mybir.AluOpType.mult,
                        )

                    hs = hp.tile([P, 8, DK], SDT, tag="hs")
                    for i in range(8):
                        pv = psC.tile([P, DK + 1], F32, tag="pv")
                        for j in range(i + 1):
                            nc.tensor.matmul(
                                pv[:],
                                mmc(pt[:, ptoff[j] + (i - j) * P : ptoff[j] + (i - j + 1) * P]),
                                mmc(vo[:, j, :]),
                                start=(j == 0),
                                stop=(j == i),
                            )
                        rec = sp.tile([P, 1], F32, tag="rec")
                        nc.vector.reciprocal(rec[:], pv[:, DK : DK + 1])
                        nc.vector.tensor_scalar_mul(hs[:, i, :], pv[:, :DK], rec[:])
                    nc.gpsimd.dma_start(op_w[h], hs[:])

            # ---------------- Phase 3: output projection --------------------
            opT = xtp.tile([P, 8, 1024], MDT, tag="xt")
            for m in range(8):
                opn = iop.tile([P, 1024], SDT, tag="opn")
                nc.sync.dma_start(opn[:], op_d.ap()[m * P : (m + 1) * P, :])
                for cc in range(8):
                    tp = psT.tile([P, P], SDT, tag="tph")
                    nc.tensor.transpose(
                        tp[:], opn[:, cc * P : (cc + 1) * P], identb[:]
                    )
                    nc.vector.tensor_copy(
                        out=opT[:, cc, m * P : (m + 1) * P], in_=tp[:]
                    )
            for ch in range(2):
                wo_sb = wpool.tile([P, 8, 512], MDT, tag="w")
                wsrc = wo_d.ap()[:, ch * 512 : (ch + 1) * 512]
                if MDT == F32:
                    nc.sync.dma_start(
                        wo_sb[:], wsrc.rearrange("(kd p) c -> p kd c", p=P)
                    )
                else:
                    for kd in range(8):
                        wstg = iop.tile([P, 512], F32, tag="wstg")
                        nc.sync.dma_start(wstg[:], wsrc[kd * P : (kd + 1) * P, :])
                        nc.scalar.copy(out=wo_sb[:, kd, :], in_=wstg[:])
                for mt in range(8):
                    ps = psA.tile([P, 512], F32, tag="mm")
                    for cd in range(8):
                        nc.tensor.matmul(
                            ps[:],
                            mmc(opT[:, cd, mt * P : (mt + 1) * P]),
                            mmc(wo_sb[:, cd, :]),
                            start=(cd == 0),
                            stop=(cd == 7),
                        )
                    stg = iop.tile([P, 512], F32, tag="stgo")
                    nc.vector.tensor_copy(out=stg[:], in_=ps[:])
                    nc.scalar.dma_start(
                        out_d.ap()[
                            mt * P : (mt + 1) * P, ch * 512 : (ch + 1) * 512
                        ],
                        stg[:],
                    )

    if not nc.is_finalized():
        nc.finalize()
    return nc


# ======================================================================
# v2: restructured bf16 kernel.
#
# Layouts (all SBUF, bf16 matmul operands, fp32 PSUM):
#   xT[p, kd, s]        = X[s, kd*128+p]            (X^T; X in {q,k,v})
#   w[p, kd, c]         = W[kd*128+p, c]            (natural W)
#   QP^T chunk (pb,ch)  = psum[c_loc*64+dk, (h-8ch)*64+r],  c = 2pb+c_loc
#   QhT[par*64+dk, hp, c, r]   = Q_h^T[dk, q'=r*16+c],  h = 2hp+par
#   KhT[par*64+dk, hp, k']     = K_h^T[dk, k']          (physical k')
#   vo[rr*16+c, j, dk]  = V_h[k'=(8j+rr)*16+c, dk]   (+ ones col at dk=64)
#   pt strips: per k-block j, pieces of (c:16)x(rsub mult of 8),
#              exp packed into nine [128,512] psum chunks per head
#   HT[par*64+dk, kd, s] = H[s, kd*128 + par*64 + dk]
#
# Scores for head h use 64-partition operands (rows par*64..par*64+64).
# Causality: k-block j covers q' >= 128j exactly (r >= 8j); the diagonal
# 128-block is fixed by a precomputed permuted mask M[p, c*8+rr].
# One-head software pipeline: scores(h+1) issue before PV(h) so the ACT
# engine (exp) never starves behind PV/projection matmuls in the PE FIFO.
# ======================================================================


def _build_nc_v2():
    nc = bacc.Bacc(
        "TRN2",
        target_bir_lowering=False,
        debug=False,
        enable_asserts=False,
        num_devices=B,
    )

    q_d = nc.dram_tensor("q", [S, D], F32, kind="ExternalInput")
    k_d = nc.dram_tensor("k", [S, D], F32, kind="ExternalInput")
    v_d = nc.dram_tensor("v", [S, D], F32, kind="ExternalInput")
    wq_d = nc.dram_tensor("w_q", [D, D], F32, kind="ExternalInput")
    wk_d = nc.dram_tensor("w_k", [D, D], F32, kind="ExternalInput")
    wv_d = nc.dram_tensor("w_v", [D, D], F32, kind="ExternalInput")
    wo_d = nc.dram_tensor("w_o", [D, D], F32, kind="ExternalInput")
    out_d = nc.dram_tensor("out", [S, D], F32, kind="ExternalOutput")
    vp_d = nc.dram_tensor("vp_scratch", [S, D], BF16, kind="Internal")

    with tile.TileContext(nc) as tc:
        with (
            tc.tile_pool(name="consts", bufs=1) as consts,
            tc.tile_pool(name="bigp", bufs=3) as bigp,
            tc.tile_pool(name="wp", bufs=3) as wp,
            tc.tile_pool(name="wsp", bufs=4) as wsp,
            tc.tile_pool(name="qtp", bufs=1) as qtp,
            tc.tile_pool(name="ktp", bufs=1) as ktp,
            tc.tile_pool(name="htp", bufs=1) as htp,
            tc.tile_pool(name="natp", bufs=3) as natp,
            tc.tile_pool(name="vstgp", bufs=1) as vstgp,
            tc.tile_pool(name="vop", bufs=2) as vop,
            tc.tile_pool(name="ptp", bufs=3) as ptp,
            tc.tile_pool(name="hsp", bufs=3) as hsp,
            tc.tile_pool(name="rp", bufs=4) as rp,
            tc.tile_pool(name="op_", bufs=1) as op_,
            tc.tile_pool(name="psA", bufs=3, space="PSUM") as psA,
            tc.tile_pool(name="psS", bufs=3, space="PSUM") as psS,
            tc.tile_pool(name="psO", bufs=1, space="PSUM") as psO,
            tc.tile_pool(name="psV", bufs=1, space="PSUM") as psV,
        ):
            ident = consts.tile([P, P], F32, tag="ident")
            make_identity(nc, ident[:])
            identb = consts.tile([P, P], BF16, tag="identb")
            make_identity(nc, identb[:])

            # physical causal mask: tri[k, q] = 1.0 if q >= k else 0.0
            tri = consts.tile([P, P], BF16, tag="tri")
            nc.gpsimd.memset(tri[:], 1.0)
            nc.gpsimd.affine_select(
                out=tri[:],
                in_=tri[:],
                compare_op=mybir.AluOpType.is_ge,
                fill=0.0,
                base=0,
                pattern=[[1, P]],
                channel_multiplier=-1,
            )

            # ---------------- helpers ----------------------------------
            def load_weight(w_d, w_sb, engines):
                # staged half-kd-tiles, cast f32 -> bf16
                for i in range(16):
                    kd, half = i // 2, i % 2
                    wstg = wsp.tile([P, 512], F32, tag="wstg")
                    nc.sync.dma_start(
                        wstg[:],
                        w_d.ap()[
                            kd * P : (kd + 1) * P, half * 512 : (half + 1) * 512
                        ],
                    )
                    eng = engines[i % len(engines)]
                    dst = w_sb[:, kd, half * 512 : (half + 1) * 512]
                    if eng is nc.scalar:
                        eng.copy(out=dst, in_=wstg[:])
                    else:
                        eng.tensor_copy(out=dst, in_=wstg[:])

            def load_transpose(x_d, xt, engines):
                # DRAM natural -> SBUF X^T (bf16), 4-packed f32 transposes
                for tp2 in range(4):
                    nat = natp.tile([P, 2, 1024], F32, tag="nat")
                    nc.sync.dma_start(
                        nat[:],
                        x_d.ap()[tp2 * 256 : (tp2 + 1) * 256, :].rearrange(
                            "(t p) c -> p t c", p=P
                        ),
                    )
                    for tl in range(2):
                        t = tp2 * 2 + tl
                        for g in range(2):
                            ps = psA.tile([P, 512], F32, tag="mm")
                            for kk in range(4):
                                kd = g * 4 + kk
                                nc.tensor.transpose(
                                    ps[:, kk * P : (kk + 1) * P],
                                    nat[:, tl, kd * P : (kd + 1) * P],
                                    ident[:],
                                )
                            src = ps[:].rearrange("p (a c) -> p a c", a=4)
                            dst = xt[:, g * 4 : (g + 1) * 4, t * P : (t + 1) * P]
                            eng = engines[(t * 2 + g) % len(engines)]
                            if eng is nc.scalar:
                                eng.copy(out=dst, in_=src)
                            else:
                                eng.tensor_copy(out=dst, in_=src)

            QhT0 = qtp.tile([P, 4, 1024], BF16, tag="qhT0")
            QhT1 = qtp.tile([P, 4, 1024], BF16, tag="qhT1")
            KhT0 = ktp.tile([P, 4, 1024], BF16, tag="khT0")
            KhT1 = ktp.tile([P, 4, 1024], BF16, tag="khT1")
            QhTs = (QhT0, QhT1)
            KhTs = (KhT0, KhT1)
            QhTs_r = tuple(
                t[:].rearrange("p hp (r c) -> p hp r c", c=16) for t in QhTs
            )
            KhTs_r = tuple(
                t[:].rearrange("p hp (r c) -> p hp r c", c=16) for t in KhTs
            )

            def proj_chunk(xt, w_sb, ch, pb, is_q):
                ps = psA.tile([P, 512], F32, tag="mm")
                for kd in range(8):
                    nc.tensor.matmul(
                        ps[:],
                        w_sb[:, kd, pb * P : (pb + 1) * P],
                        xt[:, kd, ch * 512 : (ch + 1) * 512],
                        start=(kd == 0),
                        stop=(kd == 7),
                    )
                psr = ps[:].rearrange("p (h4 two r) -> p h4 two r", h4=4, two=2)
                for c_loc in range(2):
                    c = 2 * pb + c_loc
                    for par in range(2):
                        src = psr[c_loc * 64 : (c_loc + 1) * 64, :, par, :]
                        dst = (QhTs_r if is_q else KhTs_r)[ch][
                            par * 64 : (par + 1) * 64, :, :, c
                        ]
                        if par == 0:
                            nc.vector.tensor_copy(out=dst, in_=src)
                        else:
                            nc.scalar.copy(out=dst, in_=src)

            # ---------------- Q, K ch0 projections up front -------------
            wq_sb = wp.tile([P, 8, 1024], BF16, tag="w")
            qT = bigp.tile([P, 8, 1024], BF16, tag="xt")
            load_transpose(q_d, qT, [nc.scalar])
            load_weight(wq_d, wq_sb, [nc.vector])
            for pb in range(8):
                proj_chunk(qT, wq_sb, 0, pb, True)

            wk_sb = wp.tile([P, 8, 1024], BF16, tag="w")
            kT = bigp.tile([P, 8, 1024], BF16, tag="xt")
            load_transpose(k_d, kT, [nc.scalar])
            load_weight(wk_d, wk_sb, [nc.vector, nc.gpsimd])
            for pb in range(8):
                proj_chunk(kT, wk_sb, 0, pb, False)

            wv_sb = wp.tile([P, 8, 1024], BF16, tag="w")
            vT = bigp.tile([P, 8, 1024], BF16, tag="xt")
            load_transpose(v_d, vT, [nc.vector, nc.scalar])
            load_weight(wv_d, wv_sb, [nc.scalar, nc.vector])

            vstg_tiles = {}

            def vp_group(m, ch):
                if ch == 0:
                    vstg_t = vstgp.tile([P, 1024], BF16, tag="vstg")
                    vstg_tiles[m] = vstg_t
                vstg = vstg_tiles[m]
                ps = psA.tile([P, 512], F32, tag="mm")
                for kd in range(8):
                    nc.tensor.matmul(
                        ps[:],
                        vT[:, kd, m * P : (m + 1) * P],
                        wv_sb[:, kd, ch * 512 : (ch + 1) * 512],
                        start=(kd == 0),
                        stop=(kd == 7),
                    )
                nc.vector.tensor_copy(
                    out=vstg[:, ch * 512 : (ch + 1) * 512], in_=ps[:]
                )
                if ch == 1:
                    nc.scalar.dma_start(
                        vp_d.ap()[m * P : (m + 1) * P, :], vstg[:]
                    )

            for m in range(8):
                vp_group(m, 0)
                vp_group(m, 1)
            for pb in range(8):
                proj_chunk(qT, wq_sb, 1, pb, True)
            for pb in range(8):
                proj_chunk(kT, wk_sb, 1, pb, False)

            deferred = []

            # wo: loaded now, casts kept off the ACT engine (exp) path
            wo_sb = wp.tile([P, 8, 1024], BF16, tag="w")
            load_weight(wo_d, wo_sb, [nc.gpsimd, nc.vector])

            HT0 = htp.tile([P, 8, 256], BF16, tag="ht0")
            HT1 = htp.tile([P, 8, 256], BF16, tag="ht1")
            HT2 = htp.tile([P, 8, 256], BF16, tag="ht2")
            HT3 = htp.tile([P, 8, 256], BF16, tag="ht3")
            HTs = (HT0, HT1, HT2, HT3)

            def vo_load(h):
                vo = vop.tile([P, 8, DK + 1], BF16, tag="vo")
                nc.vector.memset(vo[:, :, DK : DK + 1], 1.0)
                src = vp_d.ap()[h * 64 : (h + 1) * 64, :].rearrange(
                    "(j rr) (c u) -> (rr c) j u", j=8, c=16
                )
                nc.sync.dma_start(vo[:, :, 0:DK], src)
                return vo

            vo_tiles = {hh: vo_load(hh) for hh in range(3)}

            def final_m(m):
                t = m // 2
                if True:
                    ostg = op_.tile([P, 1024], F32, tag="ostg")
                    for ch in range(2):
                        ps = psA.tile([P, 512], F32, tag="mm")
                        for kd in range(8):
                            nc.tensor.matmul(
                                ps[:],
                                HTs[t][:, kd, (m - 2 * t) * P : (m - 2 * t + 1) * P],
                                wo_sb[:, kd, ch * 512 : (ch + 1) * 512],
                                start=(kd == 0),
                                stop=(kd == 7),
                            )
                        if ch == 0:
                            nc.vector.tensor_copy(
                                out=ostg[:, 0:512], in_=ps[:]
                            )
                        else:
                            nc.scalar.copy(
                                out=ostg[:, 512:1024], in_=ps[:]
                            )
                        # half-width store: ch0's DMA overlaps ch1's evac
                        nc.sync.dma_start(
                            out_d.ap()[
                                m * P : (m + 1) * P, ch * 512 : (ch + 1) * 512
                            ],
                            ostg[:, ch * 512 : (ch + 1) * 512],
                        )

            def attn_scores(h):
                ch, par = h // 8, h % 2
                hp_i = (h // 2) % 4
                KhT = KhTs[ch]
                QhT = QhTs[ch]
                r0, r1 = par * 64, par * 64 + 64
                pt = ptp.tile([P, 4608], BF16, tag="pt")
                pieces = []  # per j: list of (pt_base, q_off, n)
                cur, cur_fill, cur_base, pt_col = None, 0, 0, 0
                for j in range(8):
                    strip_pieces = []
                    off = 128 * j
                    while off < 1024:
                        if cur is None:
                            cur = psS.tile([P, 512], F32, tag="sc")
                            cur_fill = 0
                            cur_base = pt_col
                        n = min(512, 1024 - off, 512 - cur_fill)
                        nc.tensor.matmul(
                            cur[:, cur_fill : cur_fill + n],
                            KhT[r0:r1, hp_i, j * P : (j + 1) * P],
                            QhT[r0:r1, hp_i, off : off + n],
                            start=True,
                            stop=True,
                        )
                        strip_pieces.append((pt_col, off, n))
                        cur_fill += n
                        pt_col += n
                        off += n
                        if cur_fill == 512:
                            nc.scalar.activation(
                                out=pt[:, cur_base : cur_base + 512],
                                in_=cur[:],
                                func=mybir.ActivationFunctionType.Exp,
                                scale=0.125,
                            )
                            cur = None
                    pieces.append(strip_pieces)
                assert cur is None and pt_col == 4608
                return pt, pieces

            def apply_masks(pt, pieces):
                # diagonal causal mask (first 128 cols of each strip)
                for j in range(8):
                    base, o, n = pieces[j][0]
                    ptd = pt[:, base : base + P]
                    nc.gpsimd.tensor_tensor(ptd, ptd, tri[:], mybir.AluOpType.mult)

            def pt_block(pt, pieces, i, j):
                # piece containing q' in [128i, 128i+128)
                for base, o, n in pieces[j]:
                    if o <= 128 * i and 128 * i + P <= o + n:
                        return pt[:, base + 128 * i - o : base + 128 * i - o + P]
                raise AssertionError("no piece")

            def attn_pv(h, pt, pieces, vo, tail=False):
                hs = hsp.tile([P, 8, DK], BF16, tag="hs")
                for ig in range(2):
                    if tail and ig == 1:
                        # scores are done; a free sc bank breaks the
                        # psV ig0->ig1 serialization on the tail
                        pvt = psS.tile([P, 512], F32, tag="sc")
                        pv = pvt[:, 0 : 4 * (DK + 1)].rearrange(
                            "p (a b) -> p a b", b=DK + 1
                        )
                    else:
                        pv = psV.tile([P, 4, DK + 1], F32, tag="pv")
                    for il in range(4):
                        i = ig * 4 + il
                        for j in range(i + 1):
                            nc.tensor.matmul(
                                pv[:, il, :],
                                pt_block(pt, pieces, i, j),
                                vo[:, j, :],
                                start=(j == 0),
                                stop=(j == i),
                            )
                    rec = rp.tile([P, 4], F32, tag="rec")
                    nc.vector.reciprocal(rec[:], pv[:, :, DK : DK + 1])
                    for il in range(4):
                        i = ig * 4 + il
                        nc.vector.tensor_scalar_mul(
                            hs[:, i, :], pv[:, il, 0:DK], rec[:, il : il + 1]
                        )
                return hs

            def attn_otrans(h, hs):
                # po cols: physical q'-within-block; q'loc = 16*rl + 2*kd + par
                for g in range(2):
                    po = psO.tile([64, 512], BF16, tag="ot")
                    for il in range(4):
                        i = g * 4 + il
                        nc.tensor.transpose(
                            po[:, il * P : (il + 1) * P], hs[:, i, :], identb[:]
                        )
                    por = po[:].rearrange(
                        "p (il rl kd2 pr) -> p kd2 il rl pr", il=4, rl=8, kd2=8
                    )
                    base_s = (h % 4) * 64 + 32 * g
                    for par in range(2):
                        dst = HTs[h // 4][
                            par * 64 : (par + 1) * 64, :, base_s : base_s + 32
                        ].rearrange("p kd (il rl) -> p kd il rl", il=4)
                        if h == 15 and par == 1:
                            nc.scalar.copy(out=dst, in_=por[:, :, :, :, par])
                        else:
                            nc.vector.tensor_copy(
                                out=dst, in_=por[:, :, :, :, par]
                            )

            # ---------------- head loop (one-head software pipeline) ----
            pending_final = []
            prev = None  # (h, pt, pieces)
            for h in range(H):
                pt, pieces = attn_scores(h)
                for _ in range(4):
                    if deferred:
                        deferred.pop(0)()
                if prev is not None:
                    hprev, pt0, p0 = prev
                    apply_masks(pt0, p0)
                    hs = attn_pv(hprev, pt0, p0, vo_tiles[hprev])
                    attn_otrans(hprev, hs)
                    if hprev % 4 == 3:
                        t = hprev // 4
                        pending_final.extend([2 * t, 2 * t + 1])
                if pending_final:
                    final_m(pending_final.pop(0))
                prev = (h, pt, pieces)
                if h + 2 < H:
                    vo_tiles[h + 2] = vo_load(h + 2)  # noqa
            hprev, pt0, p0 = prev
            apply_masks(pt0, p0)
            hs = attn_pv(hprev, pt0, p0, vo_tiles[hprev], tail=True)
            attn_otrans(hprev, hs)
            for m in pending_final + [6, 7]:
                final_m(m)

    if not nc.is_finalized():
        nc.finalize()
    return nc


# ======================================================================
# v3: all loads are gpsimd cast-DMAs (f32 DRAM -> bf16 SBUF, no staging
# copies), bf16 input transposes, early head-loop start with the
# remaining projection work (Q/K ch1, V m>=2) interleaved into the head
# loop, causal masks on DVE, PV psum double-buffered, evacuations
# rebalanced across DVE/ACT/Pool so ACT does (almost) only exp.
# ======================================================================


def _build_nc_v3():
    nc = bacc.Bacc(
        "TRN2",
        target_bir_lowering=False,
        debug=False,
        enable_asserts=False,
        num_devices=B,
    )

    q_d = nc.dram_tensor("q", [S, D], F32, kind="ExternalInput")
    k_d = nc.dram_tensor("k", [S, D], F32, kind="ExternalInput")
    v_d = nc.dram_tensor("v", [S, D], F32, kind="ExternalInput")
    wq_d = nc.dram_tensor("w_q", [D, D], F32, kind="ExternalInput")
    wk_d = nc.dram_tensor("w_k", [D, D], F32, kind="ExternalInput")
    wv_d = nc.dram_tensor("w_v", [D, D], F32, kind="ExternalInput")
    wo_d = nc.dram_tensor("w_o", [D, D], F32, kind="ExternalInput")
    out_d = nc.dram_tensor("out", [S, D], F32, kind="ExternalOutput")
    vp_d = nc.dram_tensor("vp_scratch", [S, D], BF16, kind="Internal")

    with tile.TileContext(nc) as tc:
        with (
            tc.tile_pool(name="consts", bufs=1) as consts,
            tc.tile_pool(name="wp", bufs=3) as wp,
            tc.tile_pool(name="natp", bufs=4) as natp,
            tc.tile_pool(name="bigp", bufs=3) as bigp,
            tc.tile_pool(name="qtp", bufs=1) as qtp,
            tc.tile_pool(name="ktp", bufs=1) as ktp,
            tc.tile_pool(name="htp", bufs=1) as htp,
            tc.tile_pool(name="vstgp", bufs=3) as vstgp,
            tc.tile_pool(name="vop", bufs=4) as vop,
            tc.tile_pool(name="ptp", bufs=3) as ptp,
            tc.tile_pool(name="hsp", bufs=3) as hsp,
            tc.tile_pool(name="rp", bufs=4) as rp,
            tc.tile_pool(name="op_", bufs=3) as op_,
            tc.tile_pool(name="psA", bufs=2, space="PSUM") as psA,
            tc.tile_pool(name="psS", bufs=3, space="PSUM") as psS,
            tc.tile_pool(name="psV", bufs=2, space="PSUM") as psV,
            tc.tile_pool(name="psO", bufs=1, space="PSUM") as psO,
        ):
            # memsets on DVE; only the two affine_selects touch the Pool
            # queue ahead of the DMA issue burst (~0.6us)
            identb = consts.tile([P, P], BF16, tag="identb")
            nc.vector.memset(identb[:], 0.0)
            nc.gpsimd.affine_select(
                out=identb[:],
                in_=identb[:],
                compare_op=mybir.AluOpType.not_equal,
                fill=1.0,
                base=0,
                pattern=[[-1, P]],
                channel_multiplier=1,
            )

            # causal mask via matmul: scores_psum += neg240I^T @ trs adds
            # -240 where q < k (diag block); exp(0.125*(s-240)) ~ 1e-11.
            # neg240I[p, c] = -240 if p == c else 0
            neg240I = consts.tile([P, P], BF16, tag="neg240I")
            nc.vector.memset(neg240I[:], 0.0)
            nc.gpsimd.affine_select(
                out=neg240I[:],
                in_=neg240I[:],
                compare_op=mybir.AluOpType.not_equal,
                fill=-240.0,
                base=0,
                pattern=[[-1, P]],
                channel_multiplier=1,
            )
            # trs[k, q] = 1.0 if q < k else 0.0 (strictly masked region):
            # keep 0 where q >= k, fill 1 where q < k
            trs = consts.tile([P, P], BF16, tag="trs")
            nc.vector.memset(trs[:], 0.0)
            nc.gpsimd.affine_select(
                out=trs[:],
                in_=trs[:],
                compare_op=mybir.AluOpType.is_ge,
                fill=1.0,
                base=0,
                pattern=[[1, P]],
                channel_multiplier=-1,
            )

            # ---------------- load issue: gpsimd cast DMAs --------------
            wq_sb = wp.tile([P, 8, 1024], BF16, tag="w")
            wk_sb = wp.tile([P, 8, 1024], BF16, tag="w")
            wv_sb = wp.tile([P, 8, 1024], BF16, tag="w")
            wo_sb = None  # allocated late, reuses wq's buffer
            qT = bigp.tile([P, 8, 1024], BF16, tag="xt")
            kT = bigp.tile([P, 8, 1024], BF16, tag="xt")
            vT = bigp.tile([P, 8, 1024], BF16, tag="xt")

            def load_x_chunks(x_d, split_first=False):
                out = []
                for m in range(4):
                    natt = natp.tile([P, 2, 1024], BF16, tag="nat")
                    src = x_d.ap()[m * 256 : (m + 1) * 256, :].rearrange(
                        "(t p) c -> p t c", p=P
                    )
                    if m == 0 and split_first:
                        # two half-DMAs so the very first transpose can
                        # start ~0.7us earlier
                        nc.gpsimd.dma_start(natt[:, 0:1, :], src[:, 0:1, :])
                        nc.gpsimd.dma_start(natt[:, 1:2, :], src[:, 1:2, :])
                    else:
                        nc.gpsimd.dma_start(natt[:], src)
                    out.append(natt)
                return out

            def load_w_chunks(w_d, w_sb):
                # column-first halves: chunks 0-1 deliver all kd for cols
                # 0-511, so proj pb 0-3 can start after half the weight
                for c0 in (0, 512):
                    for k0 in (0, 4):
                        nc.gpsimd.dma_start(
                            w_sb[:, k0 : k0 + 4, c0 : c0 + 512],
                            w_d.ap()[
                                k0 * P : (k0 + 4) * P, c0 : c0 + 512
                            ].rearrange("(kd p) c -> p kd c", p=P),
                        )

            q_nats = load_x_chunks(q_d, split_first=True)
            load_w_chunks(wq_d, wq_sb)
            k_nats = load_x_chunks(k_d)
            load_w_chunks(wk_d, wk_sb)
            v_nats = load_x_chunks(v_d)
            load_w_chunks(wv_d, wv_sb)
            wo_holder = []

            # ---------------- helpers -----------------------------------
            def transpose_chunk(xt, natt, m, engs):
                # natt covers s rows [m*256, m*256+256); bf16 transposes
                for tl in range(2):
                    t = 2 * m + tl
                    for g in range(2):
                        ps = psS.tile([P, 512], BF16, tag="sc")
                        for kk in range(4):
                            kd = g * 4 + kk
                            nc.tensor.transpose(
                                ps[:, kk * P : (kk + 1) * P],
                                natt[:, tl, kd * P : (kd + 1) * P],
                                identb[:],
                            )
                        src = ps[:].rearrange("p (a c) -> p a c", a=4)
                        dst = xt[:, g * 4 : (g + 1) * 4, t * P : (t + 1) * P]
                        eng = engs[(t * 2 + g) % len(engs)]
                        if eng is nc.scalar:
                            eng.copy(out=dst, in_=src)
                        else:
                            eng.tensor_copy(out=dst, in_=src)

            QhT0 = qtp.tile([P, 4, 1024], BF16, tag="qhT0")
            QhT1 = qtp.tile([P, 4, 1024], BF16, tag="qhT1")
            KhT0 = ktp.tile([P, 4, 1024], BF16, tag="khT0")
            KhT1 = ktp.tile([P, 4, 1024], BF16, tag="khT1")
            QhTs_r = tuple(
                t[:].rearrange("p hp (r c) -> p hp r c", c=16)
                for t in (QhT0, QhT1)
            )
            KhTs_r = tuple(
                t[:].rearrange("p hp (r c) -> p hp r c", c=16)
                for t in (KhT0, KhT1)
            )
            KhTs = (KhT0, KhT1)
            QhTs = (QhT0, QhT1)

            def proj_chunk(xt, w_sb, ch, pb, is_q):
                ps = psA.tile([P, 512], F32, tag="sc")
                for kd in range(8):
                    nc.tensor.matmul(
                        ps[:],
                        w_sb[:, kd, pb * P : (pb + 1) * P],
                        xt[:, kd, ch * 512 : (ch + 1) * 512],
                        start=(kd == 0),
                        stop=(kd == 7),
                    )
                psr = ps[:].rearrange("p (h4 two r) -> p h4 two r", h4=4, two=2)
                for c_loc in range(2):
                    c = 2 * pb + c_loc
                    for par in range(2):
                        src = psr[c_loc * 64 : (c_loc + 1) * 64, :, par, :]
                        dst = (QhTs_r if is_q else KhTs_r)[ch][
                            par * 64 : (par + 1) * 64, :, :, c
                        ]
                        if par == 0:
                            nc.vector.tensor_copy(out=dst, in_=src)
                        else:
                            nc.scalar.copy(out=dst, in_=src)

            vstg_tiles = {}

            def vp_group(m, ch):
                if ch == 0:
                    vstg_tiles[m] = vstgp.tile(
                        [P, 1024], BF16, tag="vstg", name=f"vstg{m}"
                    )
                vstg = vstg_tiles[m]
                ps = psA.tile([P, 512], F32, tag="sc")
                for kd in range(8):
                    nc.tensor.matmul(
                        ps[:],
                        vT[:, kd, m * P : (m + 1) * P],
                        wv_sb[:, kd, ch * 512 : (ch + 1) * 512],
                        start=(kd == 0),
                        stop=(kd == 7),
                    )
                nc.vector.tensor_copy(
                    out=vstg[:, ch * 512 : (ch + 1) * 512], in_=ps[:]
                )
                if ch == 1:
                    nc.sync.dma_start(vp_d.ap()[m * P : (m + 1) * P, :], vstg[:])

            HT0 = htp.tile([P, 8, 256], BF16, tag="ht0")
            HT1 = htp.tile([P, 8, 256], BF16, tag="ht1")
            HT2 = htp.tile([P, 8, 256], BF16, tag="ht2")
            HT3 = htp.tile([P, 8, 256], BF16, tag="ht3")
            HTs = (HT0, HT1, HT2, HT3)

            def vo_load(h):
                vo = vop.tile([P, 8, DK + 1], BF16, tag="vo")
                nc.vector.memset(vo[:, :, DK : DK + 1], 1.0)
                src = vp_d.ap()[h * 64 : (h + 1) * 64, :].rearrange(
                    "(j rr) (c u) -> (rr c) j u", j=8, c=16
                )
                nc.sync.dma_start(vo[:, :, 0:DK], src)
                return vo

            def final_m(m, tail=False):
                t = m // 2
                wo_sb = wo_holder[0]
                for ch in range(2):
                    ps = psA.tile([P, 512], F32, tag="sc")
                    for kd in range(8):
                        nc.tensor.matmul(
                            ps[:],
                            HTs[t][:, kd, (m - 2 * t) * P : (m - 2 * t + 1) * P],
                            wo_sb[:, kd, ch * 512 : (ch + 1) * 512],
                            start=(kd == 0),
                            stop=(kd == 7),
                        )
                    ostg = op_.tile([P, 512], F32, tag="ostg")
                    if tail:
                        # split evac DVE/ACT + half-width stores to shorten
                        # the end-of-kernel drain
                        nc.vector.tensor_copy(out=ostg[:, 0:256], in_=ps[:, 0:256])
                        nc.scalar.copy(out=ostg[:, 256:512], in_=ps[:, 256:512])
                        for hh in range(2):
                            nc.sync.dma_start(
                                out_d.ap()[
                                    m * P : (m + 1) * P,
                                    ch * 512 + hh * 256 : ch * 512 + hh * 256 + 256,
                                ],
                                ostg[:, hh * 256 : hh * 256 + 256],
                            )
                    else:
                        nc.vector.tensor_copy(out=ostg[:], in_=ps[:])
                        nc.sync.dma_start(
                            out_d.ap()[
                                m * P : (m + 1) * P, ch * 512 : (ch + 1) * 512
                            ],
                            ostg[:],
                        )

            def attn_scores(h):
                ch, par = h // 8, h % 2
                hp_i = (h // 2) % 4
                KhT = KhTs[ch]
                QhT = QhTs[ch]
                r0, r1 = par * 64, par * 64 + 64
                pt = ptp.tile([P, 4608], BF16, tag="pt")
                pieces = []  # per j: list of (pt_base, q_off, n)
                cur, cur_fill, cur_base, pt_col = None, 0, 0, 0
                for j in range(8):
                    strip_pieces = []
                    off = 128 * j
                    while off < 1024:
                        if cur is None:
                            cur = psS.tile([P, 512], F32, tag="sc")
                            cur_fill = 0
                            cur_base = pt_col
                        is_diag = off == 128 * j
                        n = min(
                            128 if is_diag else 512,
                            1024 - off,
                            512 - cur_fill,
                        )
                        dst = cur[:, cur_fill : cur_fill + n]
                        nc.tensor.matmul(
                            dst,
                            KhT[r0:r1, hp_i, j * P : (j + 1) * P],
                            QhT[r0:r1, hp_i, off : off + n],
                            start=True,
                            stop=not is_diag,
                        )
                        if is_diag:
                            # fused causal mask: adds -240 where q < k
                            nc.tensor.matmul(
                                dst,
                                neg240I[:],
                                trs[:],
                                start=False,
                                stop=True,
                            )
                        strip_pieces.append((pt_col, off, n))
                        cur_fill += n
                        pt_col += n
                        off += n
                        if cur_fill == 512:
                            nc.scalar.activation(
                                out=pt[:, cur_base : cur_base + 512],
                                in_=cur[:],
                                func=mybir.ActivationFunctionType.Exp,
                                scale=0.125,
                            )
                            cur = None
                    pieces.append(strip_pieces)
                assert cur is None and pt_col == 4608
                return pt, pieces

            def pt_block(pt, pieces, i, j):
                # piece containing q' in [128i, 128i+128)
                for base, o, n in pieces[j]:
                    if o <= 128 * i and 128 * i + P <= o + n:
                        return pt[:, base + 128 * i - o : base + 128 * i - o + P]
                raise AssertionError("no piece")

            def attn_pv(h, pt, pieces, vo, tail=False):
                hs = hsp.tile([P, 8, DK], BF16, tag="hs")
                for ig in range(2):
                    pv = psV.tile([P, 4, DK + 1], F32, tag="pv")
                    for il in range(4):
                        i = ig * 4 + il
                        for j in range(i + 1):
                            nc.tensor.matmul(
                                pv[:, il, :],
                                pt_block(pt, pieces, i, j),
                                vo[:, j, :],
                                start=(j == 0),
                                stop=(j == i),
                            )
                    rec = rp.tile([P, 4], F32, tag="rec")
                    nc.vector.reciprocal(rec[:], pv[:, :, DK : DK + 1])
                    for il in range(4):
                        i = ig * 4 + il
                        if tail and il % 2 == 1:
                            # ACT is idle at the tail; halve the DVE chain
                            nc.scalar.mul(
                                hs[:, i, :],
                                pv[:, il, 0:DK],
                                rec[:, il : il + 1],
                            )
                        else:
                            nc.vector.tensor_scalar_mul(
                                hs[:, i, :], pv[:, il, 0:DK], rec[:, il : il + 1]
                            )
                return hs

            def attn_otrans(h, hs, tail=False):
                # po cols: physical q'-within-block; q'loc = 16*rl + 2*kd + par
                po = psO.tile([64, 1024], BF16, tag="ot")
                for i in range(8):
                    nc.tensor.transpose(
                        po[:, i * P : (i + 1) * P], hs[:, i, :], identb[:]
                    )
                por = po[:].rearrange(
                    "p (g il rl kd2 pr) -> p kd2 g il rl pr",
                    g=2,
                    il=4,
                    rl=8,
                    kd2=8,
                )
                base_s = (h % 4) * 64
                for par in range(2):
                    dst = HTs[h // 4][
                        par * 64 : (par + 1) * 64, :, base_s : base_s + 64
                    ].rearrange("p kd (g il rl) -> p kd g il rl", g=2, il=4)
                    if tail and par == 1:
                        nc.scalar.copy(out=dst, in_=por[:, :, :, :, :, par])
                    else:
                        nc.vector.tensor_copy(
                            out=dst, in_=por[:, :, :, :, :, par]
                        )

            # ---------------- phase 1 -----------------------------------
            for m in range(4):
                transpose_chunk(qT, q_nats[m], m, [nc.vector, nc.vector, nc.scalar])
            for pb in range(8):
                proj_chunk(qT, wq_sb, 0, pb, True)
            for m in range(4):
                transpose_chunk(kT, k_nats[m], m, [nc.vector, nc.vector, nc.scalar])
            for pb in range(8):
                proj_chunk(kT, wk_sb, 0, pb, False)
            for m in range(4):
                transpose_chunk(vT, v_nats[m], m, [nc.vector, nc.scalar])
            for mm_ in range(2):
                vp_group(mm_, 0)
                vp_group(mm_, 1)

            # remaining projection work, interleaved into the head loop.
            # Order matters: Q ch1 first so wq's buffer frees early for wo;
            # V groups next (vo(2m) needs vstg m stored ~2 heads ahead);
            # K ch1 last (only needed from head 8).
            def load_wo():
                w_sb = wp.tile([P, 8, 1024], BF16, tag="w", name="wo_sb")
                load_w_chunks(wo_d, w_sb)
                wo_holder.append(w_sb)

            deferred = []
            for pb in range(8):
                deferred.append(
                    lambda p=pb: proj_chunk(qT, wq_sb, 1, p, True)
                )
            deferred.append(load_wo)
            for mm_ in range(2, 8):
                for ch in range(2):
                    deferred.append(
                        lambda m=mm_, c=ch: vp_group(m, c)
                    )
            for pb in range(8):
                deferred.append(
                    lambda p=pb: proj_chunk(kT, wk_sb, 1, p, False)
                )

            vo_tiles = {hh: vo_load(hh) for hh in range(2)}

            # ---------------- head loop (one-head software pipeline) ----
            pending_final = []
            prev = None  # (h, pt, pieces)
            for h in range(H):
                pt, pieces = attn_scores(h)
                for _ in range(4):
                    if deferred:
                        deferred.pop(0)()
                if prev is not None:
                    hprev, pt0, p0 = prev
                    hs = attn_pv(hprev, pt0, p0, vo_tiles[hprev])
                    attn_otrans(hprev, hs)
                    if hprev % 4 == 3:
                        t = hprev // 4
                        pending_final.extend([2 * t, 2 * t + 1])
                if pending_final:
                    final_m(pending_final.pop(0))
                prev = (h, pt, pieces)
                if h + 2 < H:
                    vo_tiles[h + 2] = vo_load(h + 2)  # noqa
            hprev, pt0, p0 = prev
            apply_masks(pt0, p0)
            hs = attn_pv(hprev, pt0, p0, vo_tiles[hprev])
            attn_otrans(hprev, hs)
            for m in pending_final + [6, 7]:
                final_m(m)

    if not nc.is_finalized():
        nc.finalize()
    return nc


_nc_cache = {}


def _get_nc(mm_mode):
    if mm_mode not in _nc_cache:
        if mm_mode == "bf16v3":
            _nc_cache[mm_mode] = _build_nc_v3()
        elif mm_mode == "bf16v2":
            _nc_cache[mm_mode] = _build_nc_v2()
        else:
            _nc_cache[mm_mode] = _build_nc(mm_mode)
    return _nc_cache[mm_mode]


MM_MODE = "bf16v3"


def kernel(q, k, v, mask, w_q, w_k, w_v, w_o, _trace=False):
    q = np.ascontiguousarray(np.asarray(q, dtype=np.float32))
    k = np.ascontiguousarray(np.asarray(k, dtype=np.float32))
    v = np.ascontiguousarray(np.asarray(v, dtype=np.float32))
    w_q = np.ascontiguousarray(np.asarray(w_q, dtype=np.float32))
    w_k = np.ascontiguousarray(np.asarray(w_k, dtype=np.float32))
    w_v = np.ascontiguousarray(np.asarray(w_v, dtype=np.float32))
    w_o = np.ascontiguousarray(np.asarray(w_o, dtype=np.float32))

    nc = _get_nc(MM_MODE)
    in_maps = [
        {
            "q": q[i],
            "k": k[i],
            "v": v[i],
            "w_q": w_q,
            "w_k": w_k,
            "w_v": w_v,
            "w_o": w_o,
        }
        for i in range(B)
    ]
    res = run_bass_kernel_spmd(
        nc, in_maps, core_ids=list(range(B)), trace=_trace
    )
    out = np.stack([r["out"] for r in res.results], axis=0)
    if _trace:
        kernel.last_exec_time_ns = res.exec_time_ns
        kernel.last_trace = res.instructions_and_trace
    return out



# revision 51
# speedup vs baseline: 1.0003x; 1.0003x over previous
"""Multi-head attention Trainium2 kernel (B=8,S=1024,D=1024,H=16,DK=64).

Data-parallel over batch: one batch element per NeuronCore (8 cores).

Modes:
  f32    - exact baseline (DRAM-scratch structure, fp32 matmuls)
  f32r   - same structure, float32r matmuls
  bf16   - same structure, bf16 matmuls + bf16 scratch
  bf16v2 - restructured: projections emit transposed per-head layouts
           directly into SBUF (no q/k/o DRAM round trips), exp packed
           into 9x[128,512] chunks/head, one-head software pipeline.
  bf16v3 - default. v2 plus: every load is a gpsimd cast-DMA (f32 DRAM
           -> bf16 SBUF, no staging copies, half the modeled DMA time);
           bf16 input transposes; the head loop starts right after the
           Q/K ch0 projections with the remaining projection work
           (Q/K ch1, V m>=2, wo load) paced into the head loop via a
           deferred-work queue tuned so the PE stays fed through the
           ACT(exp)-bound late heads; causal masks on DVE; PV psum
           double-buffered; merged single-psum otrans; finals split
           into half-m chunks spread one-per-head; split evac + dual
           store queues on the last final to shorten the drain.
"""

import numpy as np

import concourse.bass as bass
import concourse.mybir as mybir
import concourse.tile as tile
from concourse import bacc
from concourse.bass_utils import run_bass_kernel_spmd
from concourse.masks import make_identity

B, S, D, H, DK = 8, 1024, 1024, 16, 64
P = 128
F32 = mybir.dt.float32
F32R = mybir.dt.float32r
BF16 = mybir.dt.bfloat16


HP_BUFS = 3


def _build_nc(mm_mode: str = "f32"):
    """Build the Bass program. mm_mode: 'f32' (exact), 'f32r' (fast fp32),
    or 'bf16' (all matmuls + DRAM scratch in bfloat16)."""

    if mm_mode == "bf16":
        MDT = BF16
        SDT = BF16  # DRAM scratch + per-head staging dtype
    else:
        MDT = F32R if mm_mode == "f32r" else F32
        SDT = F32

    def mmc(ap):
        return ap

    nc = bacc.Bacc(
        "TRN2",
        target_bir_lowering=False,
        debug=False,
        enable_asserts=False,
        num_devices=B,
    )

    q_d = nc.dram_tensor("q", [S, D], F32, kind="ExternalInput")
    k_d = nc.dram_tensor("k", [S, D], F32, kind="ExternalInput")
    v_d = nc.dram_tensor("v", [S, D], F32, kind="ExternalInput")
    wq_d = nc.dram_tensor("w_q", [D, D], F32, kind="ExternalInput")
    wk_d = nc.dram_tensor("w_k", [D, D], F32, kind="ExternalInput")
    wv_d = nc.dram_tensor("w_v", [D, D], F32, kind="ExternalInput")
    wo_d = nc.dram_tensor("w_o", [D, D], F32, kind="ExternalInput")
    out_d = nc.dram_tensor("out", [S, D], F32, kind="ExternalOutput")

    qp_d = nc.dram_tensor("qp_scratch", [S, D], SDT, kind="Internal")
    kp_d = nc.dram_tensor("kp_scratch", [S, D], SDT, kind="Internal")
    vp_d = nc.dram_tensor("vp_scratch", [S, D], SDT, kind="Internal")
    op_d = nc.dram_tensor("op_scratch", [S, D], SDT, kind="Internal")

    with tile.TileContext(nc) as tc:
        with (
            tc.tile_pool(name="consts", bufs=1) as consts,
            tc.tile_pool(name="wpool", bufs=2) as wpool,
            tc.tile_pool(name="xtp", bufs=1) as xtp,
            tc.tile_pool(name="iop", bufs=3) as iop,
            tc.tile_pool(name="shp", bufs=3) as shp,
            tc.tile_pool(name="hp", bufs=HP_BUFS) as hp,
            tc.tile_pool(name="ptp", bufs=3) as ptp,
            tc.tile_pool(name="sp", bufs=4) as sp,
            tc.tile_pool(name="psA", bufs=3, space="PSUM") as psA,
            tc.tile_pool(name="psB", bufs=2, space="PSUM") as psB,
            tc.tile_pool(name="psT", bufs=2, space="PSUM") as psT,
            tc.tile_pool(name="psC", bufs=2, space="PSUM") as psC,
        ):
            ident = consts.tile([P, P], F32, tag="ident")
            make_identity(nc, ident[:])
            if SDT == BF16:
                identb = consts.tile([P, P], BF16, tag="identb")
                make_identity(nc, identb[:])
            else:
                identb = ident
            # tri[k, q] = 1.0 if q >= k else 0.0  (keep causal-valid entries)
            tri = consts.tile([P, P], SDT, tag="tri")
            nc.gpsimd.memset(tri[:], 1.0)
            nc.gpsimd.affine_select(
                out=tri[:],
                in_=tri[:],
                compare_op=mybir.AluOpType.is_ge,
                fill=0.0,
                base=0,
                pattern=[[1, P]],
                channel_multiplier=-1,
            )

            # ---------------- Phase 1: projections -> DRAM scratch ----------
            for x_d, w_d, xp_d in (
                (q_d, wq_d, qp_d),
                (k_d, wk_d, kp_d),
                (v_d, wv_d, vp_d),
            ):
                xt_sb = xtp.tile([P, 8, 1024], MDT, tag="xt")
                for st in range(8):
                    nat = iop.tile([P, 1024], F32, tag="nat")
                    nc.sync.dma_start(nat[:], x_d.ap()[st * P : (st + 1) * P, :])
                    for kd in range(8):
                        tp = psB.tile([P, P], F32, tag="tp")
                        nc.tensor.transpose(
                            tp[:], nat[:, kd * P : (kd + 1) * P], ident[:]
                        )
                        nc.vector.tensor_copy(
                            out=xt_sb[:, kd, st * P : (st + 1) * P], in_=tp[:]
                        )
                for ch in range(2):
                    w_sb = wpool.tile([P, 8, 512], MDT, tag="w")
                    wsrc = w_d.ap()[:, ch * 512 : (ch + 1) * 512]
                    if MDT == F32:
                        nc.sync.dma_start(
                            w_sb[:], wsrc.rearrange("(kd p) c -> p kd c", p=P)
                        )
                    else:
                        for kd in range(8):
                            wstg = iop.tile([P, 512], F32, tag="wstg")
                            nc.sync.dma_start(
                                wstg[:], wsrc[kd * P : (kd + 1) * P, :]
                            )
                            nc.scalar.copy(out=w_sb[:, kd, :], in_=wstg[:])
                    for st in range(8):
                        ps = psA.tile([P, 512], F32, tag="mm")
                        for kd in range(8):
                            nc.tensor.matmul(
                                ps[:],
                                mmc(xt_sb[:, kd, st * P : (st + 1) * P]),
                                mmc(w_sb[:, kd, :]),
                                start=(kd == 0),
                                stop=(kd == 7),
                            )
                        stg = iop.tile([P, 512], SDT, tag="stg")
                        nc.vector.tensor_copy(out=stg[:], in_=ps[:])
                        nc.scalar.dma_start(
                            xp_d.ap()[
                                st * P : (st + 1) * P, ch * 512 : (ch + 1) * 512
                            ],
                            stg[:],
                        )

            # ------------- Phase 2: attention, one head at a time -----------
            qp_r = qp_d.ap().rearrange("(h a) (b u) -> h (a b) u", h=H, b=16)
            kp_r = kp_d.ap().rearrange("(h a) (b u) -> h (a b) u", h=H, b=16)
            vp_r = vp_d.ap().rearrange("(h a) (b u) -> h (a b) u", h=H, b=16)
            op_w = op_d.ap().rearrange(
                "(hh i pa) (pb u) -> hh pa pb i u", i=8, pa=8, pb=16
            )

            for hp2 in range(H // 2):
                h0 = 2 * hp2
                qT2 = hp.tile([P, 1024], MDT, tag="qhT")
                kT2 = hp.tile([P, 1024], MDT, tag="khT")
                qh2 = shp.tile([P, 8, P], SDT, tag="qh")
                kh2 = shp.tile([P, 8, P], SDT, tag="kh")
                for hh in range(2):
                    nc.sync.dma_start(
                        qh2[:, :, hh * DK : (hh + 1) * DK],
                        qp_r[h0 + hh].rearrange("(t p) u -> p t u", p=P),
                    )
                    nc.scalar.dma_start(
                        kh2[:, :, hh * DK : (hh + 1) * DK],
                        kp_r[h0 + hh].rearrange("(t p) u -> p t u", p=P),
                    )
                for t in range(8):
                    tpq = psT.tile([P, P], SDT, tag="tph")
                    nc.tensor.transpose(tpq[:], qh2[:, t, :], identb[:])
                    nc.vector.tensor_copy(
                        out=qT2[:, t * P : (t + 1) * P], in_=tpq[:]
                    )
                    tpk = psT.tile([P, P], SDT, tag="tph")
                    nc.tensor.transpose(tpk[:], kh2[:, t, :], identb[:])
                    nc.vector.tensor_copy(
                        out=kT2[:, t * P : (t + 1) * P], in_=tpk[:]
                    )

                for hh in range(2):
                    h = h0 + hh
                    r0, r1 = hh * DK, (hh + 1) * DK
                    vo = hp.tile([P, 8, DK + 1], SDT, tag="vo")
                    if h < HP_BUFS:
                        nc.vector.memset(vo[:, :, DK : DK + 1], 1.0)
                    nc.gpsimd.dma_start(
                        vo[:, :, :DK], vp_r[h].rearrange("(t p) u -> p t u", p=P)
                    )

                    pt = ptp.tile([P, 4608], SDT, tag="pt")
                    ptoff = [j * 1024 - 64 * j * (j - 1) for j in range(9)]
                    for j in range(8):
                        q0 = j * P
                        off = q0
                        while off < 1024:
                            n = min(512, 1024 - off)
                            ps = psA.tile([P, 512], F32, tag="mm")
                            nc.tensor.matmul(
                                ps[:, :n],
                                mmc(kT2[r0:r1, q0 : q0 + P]),
                                mmc(qT2[r0:r1, off : off + n]),
                                start=True,
                                stop=True,
                            )
                            nc.scalar.activation(
                                out=pt[:, ptoff[j] + off - q0 : ptoff[j] + off - q0 + n],
                                in_=ps[:, :n],
                                func=mybir.ActivationFunctionType.Exp,
                                scale=0.125,
                            )
                            off += n
                        nc.vector.tensor_tensor(
                            pt[:, ptoff[j] : ptoff[j] + P],
                            pt[:, ptoff[j] : ptoff[j] + P],
                            tri[:],
                            mybir.AluOpType.mult,
                        )

                    hs = hp.tile([P, 8, DK], SDT, tag="hs")
                    for i in range(8):
                        pv = psC.tile([P, DK + 1], F32, tag="pv")
                        for j in range(i + 1):
                            nc.tensor.matmul(
                                pv[:],
                                mmc(pt[:, ptoff[j] + (i - j) * P : ptoff[j] + (i - j + 1) * P]),
                                mmc(vo[:, j, :]),
                                start=(j == 0),
                                stop=(j == i),
                            )
                        rec = sp.tile([P, 1], F32, tag="rec")
                        nc.vector.reciprocal(rec[:], pv[:, DK : DK + 1])
                        nc.vector.tensor_scalar_mul(hs[:, i, :], pv[:, :DK], rec[:])
                    nc.gpsimd.dma_start(op_w[h], hs[:])

            # ---------------- Phase 3: output projection --------------------
            opT = xtp.tile([P, 8, 1024], MDT, tag="xt")
            for m in range(8):
                opn = iop.tile([P, 1024], SDT, tag="opn")
                nc.sync.dma_start(opn[:], op_d.ap()[m * P : (m + 1) * P, :])
                for cc in range(8):
                    tp = psT.tile([P, P], SDT, tag="tph")
                    nc.tensor.transpose(
                        tp[:], opn[:, cc * P : (cc + 1) * P], identb[:]
                    )
                    nc.vector.tensor_copy(
                        out=opT[:, cc, m * P : (m + 1) * P], in_=tp[:]
                    )
            for ch in range(2):
                wo_sb = wpool.tile([P, 8, 512], MDT, tag="w")
                wsrc = wo_d.ap()[:, ch * 512 : (ch + 1) * 512]
                if MDT == F32:
                    nc.sync.dma_start(
                        wo_sb[:], wsrc.rearrange("(kd p) c -> p kd c", p=P)
                    )
                else:
                    for kd in range(8):
                        wstg = iop.tile([P, 512], F32, tag="wstg")
                        nc.sync.dma_start(wstg[:], wsrc[kd * P : (kd + 1) * P, :])
                        nc.scalar.copy(out=wo_sb[:, kd, :], in_=wstg[:])
                for mt in range(8):
                    ps = psA.tile([P, 512], F32, tag="mm")
                    for cd in range(8):
                        nc.tensor.matmul(
                            ps[:],
                            mmc(opT[:, cd, mt * P : (mt + 1) * P]),
                            mmc(wo_sb[:, cd, :]),
                            start=(cd == 0),
                            stop=(cd == 7),
                        )
                    stg = iop.tile([P, 512], F32, tag="stgo")
                    nc.vector.tensor_copy(out=stg[:], in_=ps[:])
                    nc.scalar.dma_start(
                        out_d.ap()[
                            mt * P : (mt + 1) * P, ch * 512 : (ch + 1) * 512
                        ],
                        stg[:],
                    )

    if not nc.is_finalized():
        nc.finalize()
    return nc


# ======================================================================
# v2: restructured bf16 kernel.
#
# Layouts (all SBUF, bf16 matmul operands, fp32 PSUM):
#   xT[p, kd, s]        = X[s, kd*128+p]            (X^T; X in {q,k,v})
#   w[p, kd, c]         = W[kd*128+p, c]            (natural W)
#   QP^T chunk (pb,ch)  = psum[c_loc*64+dk, (h-8ch)*64+r],  c = 2pb+c_loc
#   QhT[par*64+dk, hp, c, r]   = Q_h^T[dk, q'=r*16+c],  h = 2hp+par
#   KhT[par*64+dk, hp, k']     = K_h^T[dk, k']          (physical k')
#   vo[rr*16+c, j, dk]  = V_h[k'=(8j+rr)*16+c, dk]   (+ ones col at dk=64)
#   pt strips: per k-block j, pieces of (c:16)x(rsub mult of 8),
#              exp packed into nine [128,512] psum chunks per head
#   HT[par*64+dk, kd, s] = H[s, kd*128 + par*64 + dk]
#
# Scores for head h use 64-partition operands (rows par*64..par*64+64).
# Causality: k-block j covers q' >= 128j exactly (r >= 8j); the diagonal
# 128-block is fixed by a precomputed permuted mask M[p, c*8+rr].
# One-head software pipeline: scores(h+1) issue before PV(h) so the ACT
# engine (exp) never starves behind PV/projection matmuls in the PE FIFO.
# ======================================================================


def _build_nc_v2():
    nc = bacc.Bacc(
        "TRN2",
        target_bir_lowering=False,
        debug=False,
        enable_asserts=False,
        num_devices=B,
    )

    q_d = nc.dram_tensor("q", [S, D], F32, kind="ExternalInput")
    k_d = nc.dram_tensor("k", [S, D], F32, kind="ExternalInput")
    v_d = nc.dram_tensor("v", [S, D], F32, kind="ExternalInput")
    wq_d = nc.dram_tensor("w_q", [D, D], F32, kind="ExternalInput")
    wk_d = nc.dram_tensor("w_k", [D, D], F32, kind="ExternalInput")
    wv_d = nc.dram_tensor("w_v", [D, D], F32, kind="ExternalInput")
    wo_d = nc.dram_tensor("w_o", [D, D], F32, kind="ExternalInput")
    out_d = nc.dram_tensor("out", [S, D], F32, kind="ExternalOutput")
    vp_d = nc.dram_tensor("vp_scratch", [S, D], BF16, kind="Internal")

    with tile.TileContext(nc) as tc:
        with (
            tc.tile_pool(name="consts", bufs=1) as consts,
            tc.tile_pool(name="bigp", bufs=3) as bigp,
            tc.tile_pool(name="wp", bufs=3) as wp,
            tc.tile_pool(name="wsp", bufs=4) as wsp,
            tc.tile_pool(name="qtp", bufs=1) as qtp,
            tc.tile_pool(name="ktp", bufs=1) as ktp,
            tc.tile_pool(name="htp", bufs=1) as htp,
            tc.tile_pool(name="natp", bufs=3) as natp,
            tc.tile_pool(name="vstgp", bufs=1) as vstgp,
            tc.tile_pool(name="vop", bufs=2) as vop,
            tc.tile_pool(name="ptp", bufs=3) as ptp,
            tc.tile_pool(name="hsp", bufs=3) as hsp,
            tc.tile_pool(name="rp", bufs=4) as rp,
            tc.tile_pool(name="op_", bufs=1) as op_,
            tc.tile_pool(name="psA", bufs=3, space="PSUM") as psA,
            tc.tile_pool(name="psS", bufs=3, space="PSUM") as psS,
            tc.tile_pool(name="psO", bufs=1, space="PSUM") as psO,
            tc.tile_pool(name="psV", bufs=1, space="PSUM") as psV,
        ):
            ident = consts.tile([P, P], F32, tag="ident")
            make_identity(nc, ident[:])
            identb = consts.tile([P, P], BF16, tag="identb")
            make_identity(nc, identb[:])

            # physical causal mask: tri[k, q] = 1.0 if q >= k else 0.0
            tri = consts.tile([P, P], BF16, tag="tri")
            nc.gpsimd.memset(tri[:], 1.0)
            nc.gpsimd.affine_select(
                out=tri[:],
                in_=tri[:],
                compare_op=mybir.AluOpType.is_ge,
                fill=0.0,
                base=0,
                pattern=[[1, P]],
                channel_multiplier=-1,
            )

            # ---------------- helpers ----------------------------------
            def load_weight(w_d, w_sb, engines):
                # staged half-kd-tiles, cast f32 -> bf16
                for i in range(16):
                    kd, half = i // 2, i % 2
                    wstg = wsp.tile([P, 512], F32, tag="wstg")
                    nc.sync.dma_start(
                        wstg[:],
                        w_d.ap()[
                            kd * P : (kd + 1) * P, half * 512 : (half + 1) * 512
                        ],
                    )
                    eng = engines[i % len(engines)]
                    dst = w_sb[:, kd, half * 512 : (half + 1) * 512]
                    if eng is nc.scalar:
                        eng.copy(out=dst, in_=wstg[:])
                    else:
                        eng.tensor_copy(out=dst, in_=wstg[:])

            def load_transpose(x_d, xt, engines):
                # DRAM natural -> SBUF X^T (bf16), 4-packed f32 transposes
                for tp2 in range(4):
                    nat = natp.tile([P, 2, 1024], F32, tag="nat")
                    nc.sync.dma_start(
                        nat[:],
                        x_d.ap()[tp2 * 256 : (tp2 + 1) * 256, :].rearrange(
                            "(t p) c -> p t c", p=P
                        ),
                    )
                    for tl in range(2):
                        t = tp2 * 2 + tl
                        for g in range(2):
                            ps = psA.tile([P, 512], F32, tag="mm")
                            for kk in range(4):
                                kd = g * 4 + kk
                                nc.tensor.transpose(
                                    ps[:, kk * P : (kk + 1) * P],
                                    nat[:, tl, kd * P : (kd + 1) * P],
                                    ident[:],
                                )
                            src = ps[:].rearrange("p (a c) -> p a c", a=4)
                            dst = xt[:, g * 4 : (g + 1) * 4, t * P : (t + 1) * P]
                            eng = engines[(t * 2 + g) % len(engines)]
                            if eng is nc.scalar:
                                eng.copy(out=dst, in_=src)
                            else:
                                eng.tensor_copy(out=dst, in_=src)

            QhT0 = qtp.tile([P, 4, 1024], BF16, tag="qhT0")
            QhT1 = qtp.tile([P, 4, 1024], BF16, tag="qhT1")
            KhT0 = ktp.tile([P, 4, 1024], BF16, tag="khT0")
            KhT1 = ktp.tile([P, 4, 1024], BF16, tag="khT1")
            QhTs = (QhT0, QhT1)
            KhTs = (KhT0, KhT1)
            QhTs_r = tuple(
                t[:].rearrange("p hp (r c) -> p hp r c", c=16) for t in QhTs
            )
            KhTs_r = tuple(
                t[:].rearrange("p hp (r c) -> p hp r c", c=16) for t in KhTs
            )

            def proj_chunk(xt, w_sb, ch, pb, is_q):
                ps = psA.tile([P, 512], F32, tag="mm")
                for kd in range(8):
                    nc.tensor.matmul(
                        ps[:],
                        w_sb[:, kd, pb * P : (pb + 1) * P],
                        xt[:, kd, ch * 512 : (ch + 1) * 512],
                        start=(kd == 0),
                        stop=(kd == 7),
                    )
                psr = ps[:].rearrange("p (h4 two r) -> p h4 two r", h4=4, two=2)
                for c_loc in range(2):
                    c = 2 * pb + c_loc
                    for par in range(2):
                        src = psr[c_loc * 64 : (c_loc + 1) * 64, :, par, :]
                        dst = (QhTs_r if is_q else KhTs_r)[ch][
                            par * 64 : (par + 1) * 64, :, :, c
                        ]
                        if par == 0:
                            nc.vector.tensor_copy(out=dst, in_=src)
                        else:
                            nc.scalar.copy(out=dst, in_=src)

            # ---------------- Q, K ch0 projections up front -------------
            wq_sb = wp.tile([P, 8, 1024], BF16, tag="w")
            qT = bigp.tile([P, 8, 1024], BF16, tag="xt")
            load_transpose(q_d, qT, [nc.scalar])
            load_weight(wq_d, wq_sb, [nc.vector])
            for pb in range(8):
                proj_chunk(qT, wq_sb, 0, pb, True)

            wk_sb = wp.tile([P, 8, 1024], BF16, tag="w")
            kT = bigp.tile([P, 8, 1024], BF16, tag="xt")
            load_transpose(k_d, kT, [nc.scalar])
            load_weight(wk_d, wk_sb, [nc.vector, nc.gpsimd])
            for pb in range(8):
                proj_chunk(kT, wk_sb, 0, pb, False)

            wv_sb = wp.tile([P, 8, 1024], BF16, tag="w")
            vT = bigp.tile([P, 8, 1024], BF16, tag="xt")
            load_transpose(v_d, vT, [nc.vector, nc.scalar])
            load_weight(wv_d, wv_sb, [nc.scalar, nc.vector])

            vstg_tiles = {}

            def vp_group(m, ch):
                if ch == 0:
                    vstg_t = vstgp.tile([P, 1024], BF16, tag="vstg")
                    vstg_tiles[m] = vstg_t
                vstg = vstg_tiles[m]
                ps = psA.tile([P, 512], F32, tag="mm")
                for kd in range(8):
                    nc.tensor.matmul(
                        ps[:],
                        vT[:, kd, m * P : (m + 1) * P],
                        wv_sb[:, kd, ch * 512 : (ch + 1) * 512],
                        start=(kd == 0),
                        stop=(kd == 7),
                    )
                nc.vector.tensor_copy(
                    out=vstg[:, ch * 512 : (ch + 1) * 512], in_=ps[:]
                )
                if ch == 1:
                    nc.scalar.dma_start(
                        vp_d.ap()[m * P : (m + 1) * P, :], vstg[:]
                    )

            for m in range(8):
                vp_group(m, 0)
                vp_group(m, 1)
            for pb in range(8):
                proj_chunk(qT, wq_sb, 1, pb, True)
            for pb in range(8):
                proj_chunk(kT, wk_sb, 1, pb, False)

            deferred = []

            # wo: loaded now, casts kept off the ACT engine (exp) path
            wo_sb = wp.tile([P, 8, 1024], BF16, tag="w")
            load_weight(wo_d, wo_sb, [nc.gpsimd, nc.vector])

            HT0 = htp.tile([P, 8, 256], BF16, tag="ht0")
            HT1 = htp.tile([P, 8, 256], BF16, tag="ht1")
            HT2 = htp.tile([P, 8, 256], BF16, tag="ht2")
            HT3 = htp.tile([P, 8, 256], BF16, tag="ht3")
            HTs = (HT0, HT1, HT2, HT3)

            def vo_load_pair(m):
                # both heads (2m, 2m+1) of vp block m in one DMA
                vo2 = vop.tile(
                    [P, 2, 8, DK + 1], BF16, tag="vo", name=f"vo2_{m}"
                )
                nc.vector.memset(vo2[:, :, :, DK : DK + 1], 1.0)
                src = vp_d.ap()[m * P : (m + 1) * P, :].rearrange(
                    "(hh j rr) (c u) -> (rr c) hh j u", hh=2, j=8, c=16
                )
                nc.sync.dma_start(vo2[:, :, :, 0:DK], src)
                return vo2

            def vo_load(h):
                if h % 2 == 0:
                    vo_pairs[h // 2] = vo_load_pair(h // 2)
                return vo_pairs[h // 2][:, h % 2]

            vo_pairs = {}

            vo_tiles = {hh: vo_load(hh) for hh in range(3)}

            def final_m(m):
                t = m // 2
                if True:
                    ostg = op_.tile([P, 1024], F32, tag="ostg")
                    for ch in range(2):
                        ps = psA.tile([P, 512], F32, tag="mm")
                        for kd in range(8):
                            nc.tensor.matmul(
                                ps[:],
                                HTs[t][:, kd, (m - 2 * t) * P : (m - 2 * t + 1) * P],
                                wo_sb[:, kd, ch * 512 : (ch + 1) * 512],
                                start=(kd == 0),
                                stop=(kd == 7),
                            )
                        if ch == 0:
                            nc.vector.tensor_copy(
                                out=ostg[:, 0:512], in_=ps[:]
                            )
                        else:
                            nc.scalar.copy(
                                out=ostg[:, 512:1024], in_=ps[:]
                            )
                        # half-width store: ch0's DMA overlaps ch1's evac
                        nc.sync.dma_start(
                            out_d.ap()[
                                m * P : (m + 1) * P, ch * 512 : (ch + 1) * 512
                            ],
                            ostg[:, ch * 512 : (ch + 1) * 512],
                        )

            def attn_scores(h):
                ch, par = h // 8, h % 2
                hp_i = (h // 2) % 4
                KhT = KhTs[ch]
                QhT = QhTs[ch]
                r0, r1 = par * 64, par * 64 + 64
                pt = ptp.tile([P, 4608], BF16, tag="pt")
                pieces = []  # per j: list of (pt_base, q_off, n)
                cur, cur_fill, cur_base, pt_col = None, 0, 0, 0
                for j in range(8):
                    strip_pieces = []
                    off = 128 * j
                    while off < 1024:
                        if cur is None:
                            cur = psS.tile([P, 512], F32, tag="sc")
                            cur_fill = 0
                            cur_base = pt_col
                        n = min(512, 1024 - off, 512 - cur_fill)
                        nc.tensor.matmul(
                            cur[:, cur_fill : cur_fill + n],
                            KhT[r0:r1, hp_i, j * P : (j + 1) * P],
                            QhT[r0:r1, hp_i, off : off + n],
                            start=True,
                            stop=True,
                        )
                        strip_pieces.append((pt_col, off, n))
                        cur_fill += n
                        pt_col += n
                        off += n
                        if cur_fill == 512:
                            nc.scalar.activation(
                                out=pt[:, cur_base : cur_base + 512],
                                in_=cur[:],
                                func=mybir.ActivationFunctionType.Exp,
                                scale=0.125,
                            )
                            cur = None
                    pieces.append(strip_pieces)
                assert cur is None and pt_col == 4608
                return pt, pieces

            def apply_masks(pt, pieces):
                # diagonal causal mask (first 128 cols of each strip)
                for j in range(8):
                    base, o, n = pieces[j][0]
                    ptd = pt[:, base : base + P]
                    nc.gpsimd.tensor_tensor(ptd, ptd, tri[:], mybir.AluOpType.mult)

            def pt_block(pt, pieces, i, j):
                # piece containing q' in [128i, 128i+128)
                for base, o, n in pieces[j]:
                    if o <= 128 * i and 128 * i + P <= o + n:
                        return pt[:, base + 128 * i - o : base + 128 * i - o + P]
                raise AssertionError("no piece")

            def attn_pv(h, pt, pieces, vo, tail=False):
                hs = hsp.tile([P, 8, DK], BF16, tag="hs")
                for ig in range(2):
                    if tail and ig == 1:
                        # scores are done; a free sc bank breaks the
                        # psV ig0->ig1 serialization on the tail
                        pvt = psS.tile([P, 512], F32, tag="sc")
                        pv = pvt[:, 0 : 4 * (DK + 1)].rearrange(
                            "p (a b) -> p a b", b=DK + 1
                        )
                    else:
                        pv = psV.tile([P, 4, DK + 1], F32, tag="pv")
                    for il in range(4):
                        i = ig * 4 + il
                        for j in range(i + 1):
                            nc.tensor.matmul(
                                pv[:, il, :],
                                pt_block(pt, pieces, i, j),
                                vo[:, j, :],
                                start=(j == 0),
                                stop=(j == i),
                            )
                    rec = rp.tile([P, 4], F32, tag="rec")
                    nc.vector.reciprocal(rec[:], pv[:, :, DK : DK + 1])
                    for il in range(4):
                        i = ig * 4 + il
                        nc.vector.tensor_scalar_mul(
                            hs[:, i, :], pv[:, il, 0:DK], rec[:, il : il + 1]
                        )
                return hs

            def attn_otrans(h, hs):
                # po cols: physical q'-within-block; q'loc = 16*rl + 2*kd + par
                for g in range(2):
                    po = psO.tile([64, 512], BF16, tag="ot")
                    for il in range(4):
                        i = g * 4 + il
                        nc.tensor.transpose(
                            po[:, il * P : (il + 1) * P], hs[:, i, :], identb[:]
                        )
                    por = po[:].rearrange(
                        "p (il rl kd2 pr) -> p kd2 il rl pr", il=4, rl=8, kd2=8
                    )
                    base_s = (h % 4) * 64 + 32 * g
                    for par in range(2):
                        dst = HTs[h // 4][
                            par * 64 : (par + 1) * 64, :, base_s : base_s + 32
                        ].rearrange("p kd (il rl) -> p kd il rl", il=4)
                        if h == 15 and par == 1:
                            nc.scalar.copy(out=dst, in_=por[:, :, :, :, par])
                        else:
                            nc.vector.tensor_copy(
                                out=dst, in_=por[:, :, :, :, par]
                            )

            # ---------------- head loop (one-head software pipeline) ----
            pending_final = []
            prev = None  # (h, pt, pieces)
            for h in range(H):
                pt, pieces = attn_scores(h)
                for _ in range(4):
                    if deferred:
                        deferred.pop(0)()
                if prev is not None:
                    hprev, pt0, p0 = prev
                    apply_masks(pt0, p0)
                    hs = attn_pv(hprev, pt0, p0, vo_tiles[hprev])
                    attn_otrans(hprev, hs)
                    if hprev % 4 == 3:
                        t = hprev // 4
                        pending_final.extend([2 * t, 2 * t + 1])
                if pending_final:
                    final_m(pending_final.pop(0))
                prev = (h, pt, pieces)
                if h + 2 < H:
                    vo_tiles[h + 2] = vo_load(h + 2)  # noqa
            hprev, pt0, p0 = prev
            apply_masks(pt0, p0)
            hs = attn_pv(hprev, pt0, p0, vo_tiles[hprev], tail=True)
            attn_otrans(hprev, hs)
            for m in pending_final + [6, 7]:
                final_m(m)

    if not nc.is_finalized():
        nc.finalize()
    return nc


# ======================================================================
# v3: all loads are gpsimd cast-DMAs (f32 DRAM -> bf16 SBUF, no staging
# copies), bf16 input transposes, early head-loop start with the
# remaining projection work (Q/K ch1, V m>=2) interleaved into the head
# loop, causal masks on DVE, PV psum double-buffered, evacuations
# rebalanced across DVE/ACT/Pool so ACT does (almost) only exp.
# ======================================================================


def _build_nc_v3():
    nc = bacc.Bacc(
        "TRN2",
        target_bir_lowering=False,
        debug=False,
        enable_asserts=False,
        num_devices=B,
    )

    q_d = nc.dram_tensor("q", [S, D], F32, kind="ExternalInput")
    k_d = nc.dram_tensor("k", [S, D], F32, kind="ExternalInput")
    v_d = nc.dram_tensor("v", [S, D], F32, kind="ExternalInput")
    wq_d = nc.dram_tensor("w_q", [D, D], F32, kind="ExternalInput")
    wk_d = nc.dram_tensor("w_k", [D, D], F32, kind="ExternalInput")
    wv_d = nc.dram_tensor("w_v", [D, D], F32, kind="ExternalInput")
    wo_d = nc.dram_tensor("w_o", [D, D], F32, kind="ExternalInput")
    out_d = nc.dram_tensor("out", [S, D], F32, kind="ExternalOutput")
    vp_d = nc.dram_tensor("vp_scratch", [S, D], BF16, kind="Internal")

    with tile.TileContext(nc) as tc:
        with (
            tc.tile_pool(name="consts", bufs=1) as consts,
            tc.tile_pool(name="wp", bufs=3) as wp,
            tc.tile_pool(name="natp", bufs=4) as natp,
            tc.tile_pool(name="bigp", bufs=3) as bigp,
            tc.tile_pool(name="qtp", bufs=1) as qtp,
            tc.tile_pool(name="ktp", bufs=1) as ktp,
            tc.tile_pool(name="htp", bufs=1) as htp,
            tc.tile_pool(name="vstgp", bufs=3) as vstgp,
            tc.tile_pool(name="vop", bufs=2) as vop,
            tc.tile_pool(name="ptp", bufs=3) as ptp,
            tc.tile_pool(name="hsp", bufs=3) as hsp,
            tc.tile_pool(name="rp", bufs=4) as rp,
            tc.tile_pool(name="op_", bufs=3) as op_,
            tc.tile_pool(name="psA", bufs=2, space="PSUM") as psA,
            tc.tile_pool(name="psS", bufs=3, space="PSUM") as psS,
            tc.tile_pool(name="psV", bufs=2, space="PSUM") as psV,
            tc.tile_pool(name="psO", bufs=1, space="PSUM") as psO,
        ):
            # memsets on DVE; only the two affine_selects touch the Pool
            # queue ahead of the DMA issue burst (~0.6us)
            identb = consts.tile([P, P], BF16, tag="identb")
            nc.vector.memset(identb[:], 0.0)
            nc.gpsimd.affine_select(
                out=identb[:],
                in_=identb[:],
                compare_op=mybir.AluOpType.not_equal,
                fill=1.0,
                base=0,
                pattern=[[-1, P]],
                channel_multiplier=1,
            )

            # causal mask via matmul: scores_psum += neg240I^T @ trs adds
            # -240 where q < k (diag block); exp(0.125*(s-240)) ~ 1e-11.
            # neg240I[p, c] = -240 if p == c else 0
            neg240I = consts.tile([P, P], BF16, tag="neg240I")
            nc.vector.memset(neg240I[:], 0.0)
            nc.gpsimd.affine_select(
                out=neg240I[:],
                in_=neg240I[:],
                compare_op=mybir.AluOpType.not_equal,
                fill=-240.0,
                base=0,
                pattern=[[-1, P]],
                channel_multiplier=1,
            )
            # trs[k, q] = 1.0 if q < k else 0.0 (strictly masked region):
            # keep 0 where q >= k, fill 1 where q < k
            trs = consts.tile([P, P], BF16, tag="trs")
            nc.vector.memset(trs[:], 0.0)
            nc.gpsimd.affine_select(
                out=trs[:],
                in_=trs[:],
                compare_op=mybir.AluOpType.is_ge,
                fill=1.0,
                base=0,
                pattern=[[1, P]],
                channel_multiplier=-1,
            )

            # ---------------- load issue: gpsimd cast DMAs --------------
            wq_sb = wp.tile([P, 8, 1024], BF16, tag="w")
            wk_sb = wp.tile([P, 8, 1024], BF16, tag="w")
            wv_sb = wp.tile([P, 8, 1024], BF16, tag="w")
            wo_sb = None  # allocated late, reuses wq's buffer
            qT = bigp.tile([P, 8, 1024], BF16, tag="xt")
            kT = bigp.tile([P, 8, 1024], BF16, tag="xt")
            vT = bigp.tile([P, 8, 1024], BF16, tag="xt")

            def load_x_chunks(x_d, split_first=False):
                out = []
                for m in range(4):
                    natt = natp.tile([P, 2, 1024], BF16, tag="nat")
                    src = x_d.ap()[m * 256 : (m + 1) * 256, :].rearrange(
                        "(t p) c -> p t c", p=P
                    )
                    if m == 0 and split_first:
                        # two half-DMAs so the very first transpose can
                        # start ~0.7us earlier
                        nc.gpsimd.dma_start(natt[:, 0:1, :], src[:, 0:1, :])
                        nc.gpsimd.dma_start(natt[:, 1:2, :], src[:, 1:2, :])
                    else:
                        nc.gpsimd.dma_start(natt[:], src)
                    out.append(natt)
                return out

            def load_w_chunks(w_d, w_sb):
                # column-first halves: chunks 0-1 deliver all kd for cols
                # 0-511, so proj pb 0-3 can start after half the weight
                for c0 in (0, 512):
                    for k0 in (0, 4):
                        nc.gpsimd.dma_start(
                            w_sb[:, k0 : k0 + 4, c0 : c0 + 512],
                            w_d.ap()[
                                k0 * P : (k0 + 4) * P, c0 : c0 + 512
                            ].rearrange("(kd p) c -> p kd c", p=P),
                        )

            q_nats = load_x_chunks(q_d, split_first=True)
            load_w_chunks(wq_d, wq_sb)
            k_nats = load_x_chunks(k_d)
            load_w_chunks(wk_d, wk_sb)
            v_nats = load_x_chunks(v_d)
            load_w_chunks(wv_d, wv_sb)
            wo_holder = []

            # ---------------- helpers -----------------------------------
            def transpose_chunk(xt, natt, m, engs):
                # natt covers s rows [m*256, m*256+256); bf16 transposes
                for tl in range(2):
                    t = 2 * m + tl
                    for g in range(2):
                        ps = psS.tile([P, 512], BF16, tag="sc")
                        for kk in range(4):
                            kd = g * 4 + kk
                            nc.tensor.transpose(
                                ps[:, kk * P : (kk + 1) * P],
                                natt[:, tl, kd * P : (kd + 1) * P],
                                identb[:],
                            )
                        src = ps[:].rearrange("p (a c) -> p a c", a=4)
                        dst = xt[:, g * 4 : (g + 1) * 4, t * P : (t + 1) * P]
                        eng = engs[(t * 2 + g) % len(engs)]
                        if eng is nc.scalar:
                            eng.copy(out=dst, in_=src)
                        else:
                            eng.tensor_copy(out=dst, in_=src)

            QhT0 = qtp.tile([P, 4, 1024], BF16, tag="qhT0")
            QhT1 = qtp.tile([P, 4, 1024], BF16, tag="qhT1")
            KhT0 = ktp.tile([P, 4, 1024], BF16, tag="khT0")
            KhT1 = ktp.tile([P, 4, 1024], BF16, tag="khT1")
            QhTs_r = tuple(
                t[:].rearrange("p hp (r c) -> p hp r c", c=16)
                for t in (QhT0, QhT1)
            )
            KhTs_r = tuple(
                t[:].rearrange("p hp (r c) -> p hp r c", c=16)
                for t in (KhT0, KhT1)
            )
            KhTs = (KhT0, KhT1)
            QhTs = (QhT0, QhT1)

            def proj_chunk(xt, w_sb, ch, pb, is_q):
                ps = psA.tile([P, 512], F32, tag="sc")
                for kd in range(8):
                    nc.tensor.matmul(
                        ps[:],
                        w_sb[:, kd, pb * P : (pb + 1) * P],
                        xt[:, kd, ch * 512 : (ch + 1) * 512],
                        start=(kd == 0),
                        stop=(kd == 7),
                    )
                psr = ps[:].rearrange("p (h4 two r) -> p h4 two r", h4=4, two=2)
                for c_loc in range(2):
                    c = 2 * pb + c_loc
                    for par in range(2):
                        src = psr[c_loc * 64 : (c_loc + 1) * 64, :, par, :]
                        dst = (QhTs_r if is_q else KhTs_r)[ch][
                            par * 64 : (par + 1) * 64, :, :, c
                        ]
                        if par == 0:
                            nc.vector.tensor_copy(out=dst, in_=src)
                        else:
                            nc.scalar.copy(out=dst, in_=src)

            vstg_tiles = {}

            def vp_group(m, ch):
                if ch == 0:
                    vstg_tiles[m] = vstgp.tile(
                        [P, 1024], BF16, tag="vstg", name=f"vstg{m}"
                    )
                vstg = vstg_tiles[m]
                ps = psA.tile([P, 512], F32, tag="sc")
                for kd in range(8):
                    nc.tensor.matmul(
                        ps[:],
                        vT[:, kd, m * P : (m + 1) * P],
                        wv_sb[:, kd, ch * 512 : (ch + 1) * 512],
                        start=(kd == 0),
                        stop=(kd == 7),
                    )
                nc.vector.tensor_copy(
                    out=vstg[:, ch * 512 : (ch + 1) * 512], in_=ps[:]
                )
                if ch == 1:
                    nc.sync.dma_start(vp_d.ap()[m * P : (m + 1) * P, :], vstg[:])

            HT0 = htp.tile([P, 8, 256], BF16, tag="ht0")
            HT1 = htp.tile([P, 8, 256], BF16, tag="ht1")
            HT2 = htp.tile([P, 8, 256], BF16, tag="ht2")
            HT3 = htp.tile([P, 8, 256], BF16, tag="ht3")
            HTs = (HT0, HT1, HT2, HT3)

            def vo_load_pair(m):
                # both heads (2m, 2m+1) of vp block m in one DMA
                vo2 = vop.tile(
                    [P, 2, 8, DK + 1], BF16, tag="vo", name=f"vo2_{m}"
                )
                nc.vector.memset(vo2[:, :, :, DK : DK + 1], 1.0)
                src = vp_d.ap()[m * P : (m + 1) * P, :].rearrange(
                    "(hh j rr) (c u) -> (rr c) hh j u", hh=2, j=8, c=16
                )
                nc.sync.dma_start(vo2[:, :, :, 0:DK], src)
                return vo2

            def vo_load(h):
                if h % 2 == 0:
                    vo_pairs[h // 2] = vo_load_pair(h // 2)
                return vo_pairs[h // 2][:, h % 2]

            vo_pairs = {}

            def final_m(m, tail=False):
                t = m // 2
                wo_sb = wo_holder[0]
                for ch in range(2):
                    ps = psA.tile([P, 512], F32, tag="sc")
                    for kd in range(8):
                        nc.tensor.matmul(
                            ps[:],
                            HTs[t][:, kd, (m - 2 * t) * P : (m - 2 * t + 1) * P],
                            wo_sb[:, kd, ch * 512 : (ch + 1) * 512],
                            start=(kd == 0),
                            stop=(kd == 7),
                        )
                    ostg = op_.tile([P, 512], F32, tag="ostg")
                    if tail:
                        # split evac DVE/ACT + half-width stores to shorten
                        # the end-of-kernel drain
                        nc.vector.tensor_copy(out=ostg[:, 0:256], in_=ps[:, 0:256])
                        nc.scalar.copy(out=ostg[:, 256:512], in_=ps[:, 256:512])
                        for hh in range(2):
                            nc.sync.dma_start(
                                out_d.ap()[
                                    m * P : (m + 1) * P,
                                    ch * 512 + hh * 256 : ch * 512 + hh * 256 + 256,
                                ],
                                ostg[:, hh * 256 : hh * 256 + 256],
                            )
                    else:
                        nc.vector.tensor_copy(out=ostg[:], in_=ps[:])
                        nc.sync.dma_start(
                            out_d.ap()[
                                m * P : (m + 1) * P, ch * 512 : (ch + 1) * 512
                            ],
                            ostg[:],
                        )

            def attn_scores(h):
                ch, par = h // 8, h % 2
                hp_i = (h // 2) % 4
                KhT = KhTs[ch]
                QhT = QhTs[ch]
                r0, r1 = par * 64, par * 64 + 64
                pt = ptp.tile([P, 4608], BF16, tag="pt")
                pieces = []  # per j: list of (pt_base, q_off, n)
                cur, cur_fill, cur_base, pt_col = None, 0, 0, 0
                for j in range(8):
                    strip_pieces = []
                    off = 128 * j
                    while off < 1024:
                        if cur is None:
                            cur = psS.tile([P, 512], F32, tag="sc")
                            cur_fill = 0
                            cur_base = pt_col
                        is_diag = off == 128 * j
                        n = min(
                            128 if is_diag else 512,
                            1024 - off,
                            512 - cur_fill,
                        )
                        dst = cur[:, cur_fill : cur_fill + n]
                        nc.tensor.matmul(
                            dst,
                            KhT[r0:r1, hp_i, j * P : (j + 1) * P],
                            QhT[r0:r1, hp_i, off : off + n],
                            start=True,
                            stop=not is_diag,
                        )
                        if is_diag:
                            # fused causal mask: adds -240 where q < k
                            nc.tensor.matmul(
                                dst,
                                neg240I[:],
                                trs[:],
                                start=False,
                                stop=True,
                            )
                        strip_pieces.append((pt_col, off, n))
                        cur_fill += n
                        pt_col += n
                        off += n
                        if cur_fill == 512:
                            nc.scalar.activation(
                                out=pt[:, cur_base : cur_base + 512],
                                in_=cur[:],
                                func=mybir.ActivationFunctionType.Exp,
                                scale=0.125,
                            )
                            cur = None
                    pieces.append(strip_pieces)
                assert cur is None and pt_col == 4608
                return pt, pieces

            def pt_block(pt, pieces, i, j):
                # piece containing q' in [128i, 128i+128)
                for base, o, n in pieces[j]:
                    if o <= 128 * i and 128 * i + P <= o + n:
                        return pt[:, base + 128 * i - o : base + 128 * i - o + P]
                raise AssertionError("no piece")

            def attn_pv(h, pt, pieces, vo, tail=False):
                hs = hsp.tile([P, 8, DK], BF16, tag="hs")
                for ig in range(2):
                    pv = psV.tile([P, 4, DK + 1], F32, tag="pv")
                    for il in range(4):
                        i = ig * 4 + il
                        for j in range(i + 1):
                            nc.tensor.matmul(
                                pv[:, il, :],
                                pt_block(pt, pieces, i, j),
                                vo[:, j, :],
                                start=(j == 0),
                                stop=(j == i),
                            )
                    rec = rp.tile([P, 4], F32, tag="rec")
                    nc.vector.reciprocal(rec[:], pv[:, :, DK : DK + 1])
                    for il in range(4):
                        i = ig * 4 + il
                        if tail and il % 2 == 1:
                            # ACT is idle at the tail; halve the DVE chain
                            nc.scalar.mul(
                                hs[:, i, :],
                                pv[:, il, 0:DK],
                                rec[:, il : il + 1],
                            )
                        else:
                            nc.vector.tensor_scalar_mul(
                                hs[:, i, :], pv[:, il, 0:DK], rec[:, il : il + 1]
                            )
                return hs

            def attn_otrans(h, hs, tail=False):
                # po cols: physical q'-within-block; q'loc = 16*rl + 2*kd + par
                po = psO.tile([64, 1024], BF16, tag="ot")
                for i in range(8):
                    nc.tensor.transpose(
                        po[:, i * P : (i + 1) * P], hs[:, i, :], identb[:]
                    )
                por = po[:].rearrange(
                    "p (g il rl kd2 pr) -> p kd2 g il rl pr",
                    g=2,
                    il=4,
                    rl=8,
                    kd2=8,
                )
                base_s = (h % 4) * 64
                for par in range(2):
                    dst = HTs[h // 4][
                        par * 64 : (par + 1) * 64, :, base_s : base_s + 64
                    ].rearrange("p kd (g il rl) -> p kd g il rl", g=2, il=4)
                    if tail and par == 1:
                        nc.scalar.copy(out=dst, in_=por[:, :, :, :, :, par])
                    else:
                        nc.vector.tensor_copy(
                            out=dst, in_=por[:, :, :, :, :, par]
                        )

            # ---------------- phase 1 -----------------------------------
            for m in range(4):
                transpose_chunk(qT, q_nats[m], m, [nc.vector, nc.vector, nc.scalar])
            for pb in range(8):
                proj_chunk(qT, wq_sb, 0, pb, True)
            for m in range(4):
                transpose_chunk(kT, k_nats[m], m, [nc.vector, nc.vector, nc.scalar])
            for pb in range(8):
                proj_chunk(kT, wk_sb, 0, pb, False)
            for m in range(4):
                transpose_chunk(vT, v_nats[m], m, [nc.vector, nc.scalar])
            for mm_ in range(2):
                vp_group(mm_, 0)
                vp_group(mm_, 1)

            # remaining projection work, interleaved into the head loop.
            # Order matters: Q ch1 first so wq's buffer frees early for wo;
            # V groups next (vo(2m) needs vstg m stored ~2 heads ahead);
            # K ch1 last (only needed from head 8).
            def load_wo():
                w_sb = wp.tile([P, 8, 1024], BF16, tag="w", name="wo_sb")
                load_w_chunks(wo_d, w_sb)
                wo_holder.append(w_sb)

            deferred = []
            for pb in range(8):
                deferred.append(
                    lambda p=pb: proj_chunk(qT, wq_sb, 1, p, True)
                )
            deferred.append(load_wo)
            for mm_ in range(2, 8):
                for ch in range(2):
                    deferred.append(
                        lambda m=mm_, c=ch: vp_group(m, c)
                    )
            for pb in range(8):
                deferred.append(
                    lambda p=pb: proj_chunk(kT, wk_sb, 1, p, False)
                )

            vo_tiles = {hh: vo_load(hh) for hh in range(2)}

            # ---------------- head loop (one-head software pipeline) ----
            pending_final = []
            prev = None  # (h, pt, pieces)
            for h in range(H):
                pt, pieces = attn_scores(h)
                for _ in range(4):
                    if deferred:
                        deferred.pop(0)()
                if prev is not None:
                    hprev, pt0, p0 = prev
                    hs = attn_pv(hprev, pt0, p0, vo_tiles[hprev])
                    attn_otrans(hprev, hs)
                    if hprev % 4 == 3:
                        t = hprev // 4
                        pending_final.extend([2 * t, 2 * t + 1])
                if pending_final:
                    final_m(pending_final.pop(0))
                prev = (h, pt, pieces)
                if h + 2 < H:
                    vo_tiles[h + 2] = vo_load(h + 2)  # noqa
            hprev, pt0, p0 = prev
            apply_masks(pt0, p0)
            hs = attn_pv(hprev, pt0, p0, vo_tiles[hprev])
            attn_otrans(hprev, hs)
            for m in pending_final + [6, 7]:
                final_m(m)

    if not nc.is_finalized():
        nc.finalize()
    return nc


_nc_cache = {}


def _get_nc(mm_mode):
    if mm_mode not in _nc_cache:
        if mm_mode == "bf16v3":
            _nc_cache[mm_mode] = _build_nc_v3()
        elif mm_mode == "bf16v2":
            _nc_cache[mm_mode] = _build_nc_v2()
        else:
            _nc_cache[mm_mode] = _build_nc(mm_mode)
    return _nc_cache[mm_mode]


MM_MODE = "bf16v3"


def kernel(q, k, v, mask, w_q, w_k, w_v, w_o, _trace=False):
    q = np.ascontiguousarray(np.asarray(q, dtype=np.float32))
    k = np.ascontiguousarray(np.asarray(k, dtype=np.float32))
    v = np.ascontiguousarray(np.asarray(v, dtype=np.float32))
    w_q = np.ascontiguousarray(np.asarray(w_q, dtype=np.float32))
    w_k = np.ascontiguousarray(np.asarray(w_k, dtype=np.float32))
    w_v = np.ascontiguousarray(np.asarray(w_v, dtype=np.float32))
    w_o = np.ascontiguousarray(np.asarray(w_o, dtype=np.float32))

    nc = _get_nc(MM_MODE)
    in_maps = [
        {
            "q": q[i],
            "k": k[i],
            "v": v[i],
            "w_q": w_q,
            "w_k": w_k,
            "w_v": w_v,
            "w_o": w_o,
        }
        for i in range(B)
    ]
    res = run_bass_kernel_spmd(
        nc, in_maps, core_ids=list(range(B)), trace=_trace
    )
    out = np.stack([r["out"] for r in res.results], axis=0)
    if _trace:
        kernel.last_exec_time_ns = res.exec_time_ns
        kernel.last_trace = res.instructions_and_trace
    return out

